# revision 80
# baseline (speedup 1.0000x reference)
"""
ContentAwareUpsampling (CARAFE-style) Trainium2 Bass kernel.

x[2,256,48,48] -> out[2,256,192,192]; 8 cores = 2 batches x 4 blocks of 12 rows.

Decomposition (validated bit-close by emulate() against the jax reference):
  branch convs (bf16 matmuls, BN+relu fused into ACT drains)
  softmax via exp (ACT) + group-sum/broadcast matmuls + reciprocal
  T = collapse(bilinear(softmax)) as 9 shifted matmuls vs constant [100,144]
  band matrices B built by GPSIMD local_scatter with constant indices
  einsum  E[c,(u,q,b)] = sum_dy xT[row a+dy].T @ B[dy]     (per row a, parity p)
  proj    F[r] = sum_u P_sub[u].T @ E_sigma[r*64:+64]      (sigma baked in xt)
  pixel-shuffle + BN + relu fused into the ACT drains; DMA out.
"""

import os
import numpy as np

# ---------------------------------------------------------------- constants
N, C, H, W = 2, 256, 48, 48
Cc, CK, UP, K5 = 64, 100, 2, 5
EPS = 1e-5
NCORES = 8
BLK = 12            # lo-res rows per core block
R_XIN = 20          # x rows for branch (a0-4 .. a0+15)
R_XT = 14           # x rows for einsum (a0-1 .. a0+12)
R_KN = 14           # kernel rows (a0-1 .. a0+12)
OH, OW = H * UP, W * UP          # 96
FH, FW = OH * UP, OW * UP        # 192

DYMAP = {0: [-1, -1, 0, 0, 1], 1: [-1, 0, 0, 1, 1]}
BLEND = {0: {-1: 0.25, 0: 0.75}, 1: {0: 0.75, 1: 0.25}}


def _f32(a):
    return np.ascontiguousarray(a, dtype=np.float32)


# ---------------------------------------------------------------- host prep
def _fold_bn(w, g, b, m, v):
    inv = g / np.sqrt(v + EPS)
    return w * inv[:, None], b - m * inv


def build_c2mats():
    """C2[(rho+1)*5 + (s+2)] [100, 144]: D2[b', (p,dy,q,u,dx)] =
    sum_{rho,s} Kn[:, j+rho+1, b'+s+2].T @ C2, with tau = s + dx."""
    cm = np.zeros((15, CK, 144), dtype=np.float32)
    for p in range(2):
        for q in range(2):
            for u in range(4):
                for ky in range(K5):
                    dy = DYMAP[p][ky]
                    for kx in range(K5):
                        dx = DYMAP[q][kx]
                        col = p * 72 + (dy + 1) * 24 + q * 12 + u * 3 + (dx + 1)
                        row = u * 25 + ky * 5 + kx
                        for rho, br in BLEND[p].items():
                            for tau, bc in BLEND[q].items():
                                s = tau - dx
                                w = (rho + 1) * 5 + (s + 2)
                                cm[w, row, col] += br * bc
    return cm


def build_cm2():
    """cm2[p, s'] [100, 72]: D2_p[b', (dy,q,u,dx)] =
    sum_{s'} Kb_p[:, j, b'+s'].T @ cm2[p, s'], where Kb_p holds the
    rho-blended kernel rows (blend folded on DVE) and tau = s' - 2 + dx."""
    cm = np.zeros((2, 5, CK, 72), dtype=np.float32)
    for p in range(2):
        for q in range(2):
            for u in range(4):
                for ky in range(K5):
                    dy = DYMAP[p][ky]
                    for kx in range(K5):
                        dx = DYMAP[q][kx]
                        col = (dy + 1) * 24 + q * 12 + u * 3 + (dx + 1)
                        row = u * 25 + ky * 5 + kx
                        for tau, bc in BLEND[q].items():
                            s = tau - dx
                            cm[p, s + 2, row, col] += bc
    return cm


# einsum-u -> B column block. Block k = up*2 + g where u = 2g + up, so the
# PSUM einsum output comes out ordered (up, g, q, b) and the proj repack can
# slice by u-parity with plain partition-offset copies.
U2BLK = [0, 2, 1, 3]


def build_scat_idx():
    """Scatter index set for one [80, 192] B half-tile per (j, p, side).
    Partition rho = dy*26 + w holds x column (cbase + w - 1); its source
    data is the dy-replicated D2 p-block [80, 72] with cols (dy', q, u,
    dx'); entries with dy' != dy (or out-of-window b) are -1 (ignored).
    Output cols are (ublk4, q2, bl24) with bl = w - 1 - dx."""
    sidx = np.full((80, 72), -1, dtype=np.int16)
    for dy in range(3):
        for w in range(26):
            rho = dy * 26 + w
            for q in range(2):
                for u in range(4):
                    for dx_i in range(3):
                        t = dy * 24 + q * 12 + u * 3 + dx_i
                        bl = w - dx_i
                        if 0 <= bl < 24:
                            sidx[rho, t] = U2BLK[u] * 48 + q * 24 + bl
    return sidx


def host_prep(inputs):
    x = _f32(inputs["x"])
    W1, bias1 = _fold_bn(_f32(inputs["compress_w"])[:, :, 0, 0],
                         _f32(inputs["g1"]), _f32(inputs["b1"]),
                         _f32(inputs["m1"]), _f32(inputs["v1"]))
    invk = _f32(inputs["gk"]) / np.sqrt(_f32(inputs["vk"]) + EPS)
    W2 = _f32(inputs["ke1_w"]) * invk[:, None, None, None]
    biask = _f32(inputs["bk"]) - _f32(inputs["mk"]) * invk
    W3 = _f32(inputs["ke2_w"])
    P, bias2 = _fold_bn(_f32(inputs["proj_w"])[:, :, 0, 0],
                        _f32(inputs["g2"]), _f32(inputs["b2"]),
                        _f32(inputs["m2"]), _f32(inputs["v2"]))
    cmats = build_c2mats()
    cm2 = build_cm2()
    sidx = build_scat_idx()
    # dy-replication selector: D2rep[dy*26+w', :] = D2[w' + 24*side, :]
    srep = np.zeros((50, 2, 78), dtype=np.float32)
    for side in range(2):
        for dy in range(3):
            for w in range(26):
                srep[w + 24 * side, side, dy * 26 + w] = 1.0
    # sigma position p = r*64 + m  ->  channel c = 4m + r
    sigma = np.array([4 * (p % 64) + p // 64 for p in range(256)])
    return dict(x=x, W1=W1, bias1=bias1, W2=W2, biask=biask, W3=W3,
                P=P, bias2=bias2, cmats=cmats, cm2=cm2, sidx=sidx,
                srep=srep, sigma=sigma)


def core_slices(hp, core):
    n, blk = core // 4, core % 4
    a0 = blk * BLK
    x = hp["x"][n]

    xin = np.zeros((2, 128, R_XIN, 52), dtype=np.float32)
    for s in range(R_XIN):
        row = a0 - 4 + s
        if 0 <= row < H:
            xin[0, :, s, 2:50] = x[:128, row, :]
            xin[1, :, s, 2:50] = x[128:, row, :]

    # xt kept for the numpy emulation path only
    xs = x[hp["sigma"]]
    xt = np.zeros((50, R_XT, 2, 128), dtype=np.float32)
    for s in range(R_XT):
        row = a0 - 1 + s
        if 0 <= row < H:
            xt[1:49, s, 0, :] = xs[:128, row, :].T
            xt[1:49, s, 1, :] = xs[128:, row, :].T

    # xt3: dy-replicated band windows. Partition dy*26 + w holds x column
    # (cbase(side) + w - 1) of lo-res row a0 + j - 1 + dy (0 if OOB).
    xt3 = np.zeros((80, BLK, 2, 2, 128), dtype=np.float32)
    for dy in range(3):
        for w in range(26):
            rho = dy * 26 + w
            for j in range(BLK):
                row = a0 + j - 1 + dy
                if not (0 <= row < H):
                    continue
                for side in range(2):
                    xcol = (side * 24) + w - 1
                    if 0 <= xcol < W:
                        xt3[rho, j, side, 0, :] = xs[:128, row, xcol]
                        xt3[rho, j, side, 1, :] = xs[128:, row, xcol]

    e_top = 1.0 if a0 == 0 else 0.0
    e_bot = 1.0 if a0 + BLK == H else 0.0
    mask_comp = np.array([1.0 if 0 <= a0 - 4 + s < H else 0.0
                          for s in range(R_XIN)], dtype=np.float32)
    mask_k1 = np.array([1.0 if 0 <= a0 - 2 + s < H else 0.0
                        for s in range(16)], dtype=np.float32)
    return dict(a0=a0, n=n, xin=xin, xt=xt, xt3=xt3, e_top=e_top,
                e_bot=e_bot, mask_comp=mask_comp, mask_k1=mask_k1)


# ---------------------------------------------------------------- emulation
def emulate_core(hp, cs):
    xin, xt = cs["xin"], cs["xt"]
    W1, W2, W3 = hp["W1"], hp["W2"], hp["W3"]

    xin_flat = np.concatenate([xin[0], xin[1]], axis=0)
    comp = np.zeros((Cc, R_XIN, 52), dtype=np.float32)
    comp[:, :, 2:50] = np.maximum(
        np.einsum("oc,csw->osw", W1, xin_flat[:, :, 2:50])
        + hp["bias1"][:, None, None], 0.0)
    comp *= cs["mask_comp"][None, :, None]

    k1 = np.zeros((Cc, 16, 50), dtype=np.float32)
    acc = np.zeros((Cc, 16, 48), dtype=np.float32)
    for ky in range(3):
        for kx in range(3):
            sh = comp[:, ky * 2: ky * 2 + 16, kx * 2: kx * 2 + 48]
            acc += np.einsum("oi,isw->osw", W2[:, :, ky, kx], sh)
    k1[:, :, 1:49] = np.maximum(acc + hp["biask"][:, None, None], 0.0)
    k1 *= cs["mask_k1"][None, :, None]

    kc = np.zeros((CK, R_KN, 48), dtype=np.float32)
    for ky in range(3):
        for kx in range(3):
            kc += np.einsum("oi,isw->osw", W3[:, :, ky, kx],
                            k1[:, ky: ky + 14, kx: kx + 48])

    e = np.exp(kc.reshape(4, 25, R_KN, 48))
    kn_int = (e / e.sum(axis=1, keepdims=True)).reshape(CK, R_KN, 48)

    # Kn [100, 14, 54]: col i <-> kernel pixel b = i - 3; interior i 3..50
    Kn = np.zeros((CK, R_KN, 54), dtype=np.float32)
    Kn[:, :, 3:51] = kn_int
    Kn[:, :, 2] = Kn[:, :, 3]
    Kn[:, :, 51] = Kn[:, :, 50]
    if cs["e_top"] == 1.0:
        Kn[:, 0, :] = Kn[:, 1, :]
    if cs["e_bot"] == 1.0:
        Kn[:, 13, :] = Kn[:, 12, :]

    cmats, P = hp["cmats"], hp["P"]
    scat_idx = np.full((64, 72), -1, dtype=np.int16)
    for bp in range(50):
        for dy_i in range(3):
            for q in range(2):
                for u in range(4):
                    for dx_i in range(3):
                        t = dy_i * 24 + q * 12 + u * 3 + dx_i
                        b = bp - 1 - (dx_i - 1)
                        if 0 <= b < 48:
                            scat_idx[bp, t] = dy_i * 384 + u * 96 + q * 48 + b
    out = np.zeros((C, 4 * BLK, FW), dtype=np.float32)
    for j in range(BLK):
        # D2 [50, (p, dy, q, u, dx)]
        D2 = np.zeros((50, 144), dtype=np.float32)
        for rho in (-1, 0, 1):
            for s in (-2, -1, 0, 1, 2):
                w = (rho + 1) * 5 + (s + 2)
                D2 += Kn[:, j + rho + 1, s + 2: s + 52].T @ cmats[w]
        for p in range(2):
            D = np.zeros((64, 72), dtype=np.float32)
            D[0:50] = D2[:, p * 72:(p + 1) * 72]
            B = np.zeros((64, 1152), dtype=np.float32)
            for bp in range(64):
                for t in range(72):
                    ix = scat_idx[bp, t]
                    if ix >= 0:
                        B[bp, ix] = D[bp, t]
            E = np.zeros((256, 384), dtype=np.float32)
            for ch in range(2):
                acc = np.zeros((128, 384), dtype=np.float32)
                for dy_i in range(3):
                    xsl = xt[0:50, j + dy_i, ch, :]
                    acc += xsl.T @ B[0:50, dy_i * 384:(dy_i + 1) * 384]
                E[ch * 128:(ch + 1) * 128] = acc
            for r in range(4):
                u1, u2 = r // 2, r % 2
                F = np.zeros((C, 96), dtype=np.float32)
                for u in range(4):
                    F += P[:, u * 64:(u + 1) * 64] @ \
                        E[r * 64:(r + 1) * 64, u * 96:(u + 1) * 96]
                F = np.maximum(F + hp["bias2"][:, None], 0.0)
                Y = 4 * j + 2 * p + u1
                Fq = F.reshape(C, 2, 48)
                for q in range(2):
                    out[:, Y, u2 + 2 * q::4] = Fq[:, q, :]
    return out


def emulate(inputs):
    hp = host_prep(inputs)
    out = np.zeros((N, C, FH, FW), dtype=np.float32)
    for core in range(NCORES):
        cs = core_slices(hp, core)
        n, a0 = cs["n"], cs["a0"]
        out[n, :, 4 * a0: 4 * a0 + 48, :] = emulate_core(hp, cs)
    return out


# ---------------------------------------------------------------- device
_CACHE = {}

INPUT_SPECS = [
    # DMA issue order == list order; w1t first so the PE's ldweights can
    # start the p-state ramp while xin is still in flight
    ("w1t", [128, 2, Cc], "bf16"),
    ("bias1", [Cc, 1], "f32"),
    ("xin", [128, 2, R_XIN, 52], "bf16"),
    ("mask1", [Cc, 8, 52], "bf16"),
    ("ke1t", [Cc, 9, Cc], "bf16"),
    ("biask", [Cc, 1], "f32"),
    ("mask2", [Cc, 4, 50], "bf16"),
    ("ke2t", [Cc, 9, CK], "bf16"),
    ("gden", [CK, 4], "bf16"),
    ("g2", [4, CK], "bf16"),
    ("qa", [CK, 2], "f32"),
    ("ebot", [CK, 2], "f32"),
    ("cm2", [CK, 10, 72], "bf16"),
    ("srep", [50, 2, 78], "bf16"),
    ("sidx", [80, 72], "i16"),
    ("xt3", [80, BLK, 2, 2, 128], "bf16"),
    ("bias2", [128, 2, 1], "f32"),
    ("projt", [128, 2, 2, 128], "bf16"),
]


def device_kernel(tc, outs, ins):
    """Emit the kernel. outs/ins: dicts name -> bass.AP (DRAM)."""
    from contextlib import ExitStack
    import concourse.bass as bass
    from concourse import mybir

    f32 = mybir.dt.float32
    bf16 = mybir.dt.bfloat16
    i16 = mybir.dt.int16
    AF = mybir.ActivationFunctionType
    ALU = mybir.AluOpType
    nc = tc.nc
    d_out = outs["out"]

    with ExitStack() as ctx:
        sing = ctx.enter_context(tc.tile_pool(name="sing", bufs=1))

        # ---- load inputs/constants (critical-path first, bulky ones later)
        sb = {}
        dts = {"bf16": bf16, "f32": f32, "i16": i16}
        for name, shape, dt in INPUT_SPECS:
            t = sing.tile(shape[:], dts[dt], tag=name)
            sb[name] = t
            if name == "xt3":
                nc.sync.dma_start(t[:, 0:2], ins[name][:, 0:2])
                nc.sync.dma_start(t[:, 2:6], ins[name][:, 2:6])
                nc.sync.dma_start(t[:, 6:12], ins[name][:, 6:12])
            elif name == "xin":
                # split by c-half: conv1's first matmul only needs c=0
                nc.sync.dma_start(t[:, 0], ins[name][:, 0])
                nc.sync.dma_start(t[:, 1], ins[name][:, 1])
            else:
                nc.sync.dma_start(t[:], ins[name])

        # persistent activations. All memsets go to the (otherwise idle at
        # start) GPSIMD engine, and only cover the pad regions the compute
        # never writes.
        dsbs = [sing.tile([80, 144], bf16, name=f"dsbr{i}") for i in range(4)]
        for t in dsbs:
            nc.gpsimd.memset(t[:], 0.0)
        dsb50s = [sing.tile([50, 144], bf16, name=f"dsb50r{i}")
                  for i in range(4)]
        comp = sing.tile([Cc, R_XIN, 52], bf16)
        k1 = sing.tile([Cc, 16, 50], bf16)
        exps = sing.tile([CK, R_KN, 48], bf16)
        recs = sing.tile([4, R_KN, 48], bf16)
        kn = sing.tile([CK, R_KN, 54], bf16)
        kb = sing.tile([CK, 2, BLK, 54], bf16)
        t75 = sing.tile([CK, BLK, 54], bf16)
        tb = sing.tile([CK, 54], bf16)
        nc.gpsimd.memset(comp[:, :, 0:2], 0.0)
        nc.gpsimd.memset(comp[:, :, 50:52], 0.0)
        nc.gpsimd.memset(k1[:, :, 0:1], 0.0)
        nc.gpsimd.memset(k1[:, :, 49:50], 0.0)
        nc.gpsimd.memset(kn[:, :, 0:3], 0.0)
        nc.gpsimd.memset(kn[:, :, 51:54], 0.0)

        def emit_kb(jr):
            # rho-blend fold: kb[p=0,j] = .25 kn[j] + .75 kn[j+1]
            #                 kb[p=1,j] = .75 kn[j+1] + .25 kn[j+2]
            lo, hi = jr.start, jr.stop
            nc.scalar.mul(t75[:, jr, :], kn[:, lo + 1:hi + 1, :], 0.75)
            nc.vector.scalar_tensor_tensor(
                kb[:, 0, jr, :], kn[:, lo:hi, :], 0.25,
                t75[:, jr, :], op0=ALU.mult, op1=ALU.add)
            nc.vector.scalar_tensor_tensor(
                kb[:, 1, jr, :], kn[:, lo + 2:hi + 2, :], 0.25,
                t75[:, jr, :], op0=ALU.mult, op1=ALU.add)

        # pools used by emit_tdb (created early; 1 PSUM bank + branch's 6)
        bpool = ctx.enter_context(tc.tile_pool(name="bb", bufs=6))
        pp_t = ctx.enter_context(tc.tile_pool(name="pp_t", bufs=1, space="PSUM"))
        bts = {}

        def emit_t(j):
            # T collapse for row j: D2 [50, 144] -> bf16 dsb50 ring
            pst = pp_t.tile([50, 144], f32, tag="t", name=f"pst{j}")
            for p in range(2):
                for s in range(5):
                    nc.tensor.matmul(
                        pst[:, p * 72:(p + 1) * 72], kb[:, p, j, s: s + 50],
                        sb["cm2"][:, p * 5 + s, :],
                        start=(s == 0), stop=(s == 4))
            nc.vector.tensor_copy(dsb50s[j % 4][:], pst[:])

        def emit_b(j):
            # dy-replication via a constant 0/1 selection matmul (the
            # matmul weights AP must be 1-D, so a [dy3, w26] kb slice is
            # not allowed), then one band scatter per (p, side)
            psr = pp_t.tile([78, 2, 144], f32, tag="r", name=f"psr{j}")
            for side in range(2):
                nc.tensor.matmul(psr[:, side, :], sb["srep"][:, side, :],
                                 dsb50s[j % 4][:], start=True, stop=True)
                dsb = dsbs[(2 * j + side) % 4]
                if side == 0:
                    nc.scalar.copy(dsb[0:78, :], psr[:, side, :])
                else:
                    nc.vector.tensor_copy(dsb[0:78, :], psr[:, side, :])
                for p in range(2):
                    bt = bpool.tile([80, 192], bf16, tag="b",
                                    name=f"bt{j}_{p}_{side}")
                    nc.gpsimd.local_scatter(
                        bt[:], dsb[:, p * 72:(p + 1) * 72],
                        sb["sidx"][:], channels=80,
                        num_elems=192, num_idxs=72)
                    bts[(j, p, side)] = bt

        def emit_tdb(j):
            emit_t(j)
            emit_b(j)

        # ---- branch phase
        # chunked so early rows flow to the main loop ASAP:
        #   comp rows 0:7 | 7:14 | 14:20 ; k1 rows 0:10 | 10:16 ;
        #   kn rows 0:8 | 8:14
        with tc.tile_pool(name="brp", bufs=2, space="PSUM") as brp:
            for h, (c0, CL) in enumerate(((0, 7), (7, 7), (14, 6))):
                ps = brp.tile([Cc, CL * 48], f32, tag="psc")
                for c in range(2):
                    nc.tensor.matmul(
                        ps[:], sb["w1t"][:, c, :],
                        sb["xin"][:, c, c0:c0 + CL, 2:50],
                        start=(c == 0), stop=(c == 1))
                rs = slice(c0, c0 + CL)
                if h == 0:
                    nc.vector.tensor_scalar(
                        comp[:, rs, 2:50],
                        ps[:].rearrange("c (r w) -> c r w", w=48),
                        sb["bias1"][:], 0.0, op0=ALU.add, op1=ALU.max)
                else:
                    nc.scalar.activation(
                        comp[:, rs, 2:50],
                        ps[:].rearrange("c (r w) -> c r w", w=48),
                        AF.Relu, bias=sb["bias1"][:])
                # only rows 0:4 (top core) / 16:20 (bottom core) can be OOB
                if h == 0:
                    nc.vector.tensor_mul(comp[:, 0:4, :], comp[:, 0:4, :],
                                         sb["mask1"][:, 0:4, :])
                elif h == 2:
                    nc.vector.tensor_mul(comp[:, 16:20, :],
                                         comp[:, 16:20, :],
                                         sb["mask1"][:, 4:8, :])
            for r0, L in ((0, 10), (10, 6)):
                ps = brp.tile([Cc, L * 48], f32, tag="psk1")
                for ky in range(3):
                    for kx in range(3):
                        t = ky * 3 + kx
                        nc.tensor.matmul(
                            ps[:], sb["ke1t"][:, t, :],
                            comp[:, r0 + ky * 2: r0 + ky * 2 + L,
                                 kx * 2: kx * 2 + 48],
                            start=(t == 0), stop=(t == 8))
                rs = slice(r0, r0 + L)
                nc.vector.tensor_scalar(
                    k1[:, rs, 1:49],
                    ps[:].rearrange("c (r w) -> c r w", w=48),
                    sb["biask"][:], 0.0, op0=ALU.add, op1=ALU.max)
                # only rows 0:2 (top core) / 14:16 (bottom core) can be OOB
                mr = slice(0, 2) if r0 == 0 else slice(14, 16)
                ms = slice(0, 2) if r0 == 0 else slice(2, 4)
                nc.vector.tensor_mul(k1[:, mr, :], k1[:, mr, :],
                                     sb["mask2"][:, ms, :])
            for ci, (r0, L) in enumerate(((0, 8), (8, 6))):
                rs = slice(r0, r0 + L)
                ps = brp.tile([CK, L * 48], f32, tag="psk2", bufs=2)
                for ky in range(3):
                    for kx in range(3):
                        t = ky * 3 + kx
                        nc.tensor.matmul(
                            ps[:], sb["ke2t"][:, t, :],
                            k1[:, r0 + ky: r0 + ky + L, kx: kx + 48],
                            start=(t == 0), stop=(t == 8))
                nc.scalar.activation(
                    exps[:, rs, :],
                    ps[:].rearrange("c (r w) -> c r w", w=48), AF.Exp)
                psd = brp.tile([4, L * 48], f32, tag="psc")
                nc.tensor.matmul(psd[:], sb["gden"][:], exps[:, rs, :],
                                 start=True, stop=True)
                with nc.allow_low_precision(reason="bf16 softmax pipeline"):
                    nc.vector.reciprocal(
                        recs[:, rs, :],
                        psd[:].rearrange("c (r w) -> c r w", w=48))
                psb = brp.tile([CK, L * 48], f32, tag="psc")
                nc.tensor.matmul(psb[:], sb["g2"][:], recs[:, rs, :],
                                 start=True, stop=True)
                nc.vector.tensor_mul(
                    kn[:, rs, 3:51], exps[:, rs, :],
                    psb[:].rearrange("c (r w) -> c r w", w=48))
                nc.vector.tensor_copy(kn[:, rs, 2:3], kn[:, rs, 3:4])
                nc.vector.tensor_copy(kn[:, rs, 51:52], kn[:, rs, 50:51])
                if ci == 0:
                    # kb rows 0/1 with the top-edge row blend folded into
                    # row 0's coefficients: kb[0,0] = a*kn0 + b*kn1 where
                    # a = .25*(1-e_top), b = .75 + .25*e_top
                    nc.scalar.mul(t75[:, 0:2, :], kn[:, 1:3, :], 0.75)
                    nc.scalar.mul(tb[:], kn[:, 1, :], sb["qa"][:, 1:2])
                    nc.vector.scalar_tensor_tensor(
                        kb[:, 0, 0, :], kn[:, 0, :], sb["qa"][:, 0:1],
                        tb[:], op0=ALU.mult, op1=ALU.add)
                    nc.vector.scalar_tensor_tensor(
                        kb[:, 0, 1, :], kn[:, 1, :], 0.25,
                        t75[:, 1, :], op0=ALU.mult, op1=ALU.add)
                    nc.vector.scalar_tensor_tensor(
                        kb[:, 1, 0:2, :], kn[:, 2:4, :], 0.25,
                        t75[:, 0:2, :], op0=ALU.mult, op1=ALU.add)
                    emit_kb(slice(1, 6))
                    # start the T chains for the first rows while the
                    # rest of the branch still runs
                    emit_t(0)
                    emit_t(1)
                    emit_t(2)
                    emit_t(3)
                    emit_b(0)
                    emit_b(1)
                else:
                    nc.vector.tensor_scalar_mul(kn[:, 13, :], kn[:, 13, :],
                                                sb["ebot"][:, 1:2])
                    nc.vector.scalar_tensor_tensor(
                        kn[:, 13, :], kn[:, 12, :], sb["ebot"][:, 0:1],
                        kn[:, 13, :], op0=ALU.mult, op1=ALU.add)

        # ---- main loop
        epool = ctx.enter_context(tc.tile_pool(name="esb", bufs=3))
        eapool = ctx.enter_context(tc.tile_pool(name="ea", bufs=2))
        spool = ctx.enter_context(tc.tile_pool(name="stage", bufs=3))
        pp_e = ctx.enter_context(tc.tile_pool(name="pp_e", bufs=2, space="PSUM"))
        pp_f = ctx.enter_context(tc.tile_pool(name="pp_f", bufs=2, space="PSUM"))

        esb = {}
        stage = {}

        emit_kb(slice(6, 12))
        for apair in range(BLK // 2):
            j0 = 2 * apair
            # esb: [c-half, a2, up2, g2, p2, q2, b48]
            # esb: [c-half, a2, up2, g2, p2, side2, q2, bl24]
            esb = {c: epool.tile([128, 2, 2, 2, 2, 2, 2, 24], bf16,
                                 tag=f"e{c}", name=f"esb{c}")
                   for c in range(2)}
            # [c, u1, a2, p, b, q, u2]: u1 outermost so each DMA
            # slice is one contiguous 768-element run per partition
            # stage: [c, u1, a2, p2, side2, q2, bl24, u2]
            stage = {m: spool.tile([128, 2, 2, 2, 2, 2, 24, 2], bf16,
                                   tag=f"s{m}", name=f"stage{m}")
                     for m in range(2)}
            for j in (j0, j0 + 1):
                btp = {(p, side): bts.pop((j, p, side))
                       for p in range(2) for side in range(2)}
                for c in range(2):
                    # one PSUM tile covers both parities -> one wide drain.
                    # Each (p, side) matmul writes a contiguous 192-col
                    # region (matmul PSUM outputs must not be strided);
                    # the drain's AP permutation restores esb's layout.
                    pse = pp_e.tile([128, 2, 512], f32, tag="e")
                    for p in range(2):
                        for side in range(2):
                            nc.tensor.matmul(
                                pse[:, p, side * 192:side * 192 + 192],
                                sb["xt3"][0:78, j, side, c, :],
                                btp[(p, side)][0:78, :],
                                start=True, stop=True)
                    if c == 0:
                        # DVE's table-driven lowering accepts the 4-D
                        # pattern; one wide drain
                        nc.vector.tensor_copy(
                            esb[c][:, j % 2],
                            pse[:, :, 0:384].rearrange(
                                "c p (side up g q bl) -> c up g p side q bl",
                                side=2, up=2, g=2, q=2))
                    else:
                        # ACT codegen is limited to 3-D free patterns;
                        # split by parity
                        for p in range(2):
                            nc.scalar.copy(
                                esb[c][:, j % 2, :, :, p],
                                pse[:, p, 0:384].rearrange(
                                    "c (side up g q bl) -> c up g side q bl",
                                    side=2, up=2, g=2, q=2))
                # staggered T/B prefetch: B for j+2 (whose T ran last
                # apair, so its drain has landed), T for j+4. Keeps
                # independent PE runway between einsum and proj.
                if j + 2 < BLK:
                    emit_b(j + 2)
                if j + 4 < BLK:
                    emit_t(j + 4)

            # repack + proj, r-interleaved: proj(r) starts as soon as its
            # own two repack copies land. eA[r] rows up*64+m hold channel
            # (4m+r) at einsum-u = 2g+up (g in the free dim); the copies
            # are plain partition-offset moves that DVE runs in 4x mode.
            for r in range(4):
                ea = eapool.tile([128, 2, 2, 2, 2, 2, 24], bf16,
                                 tag=f"ea{r}", name=f"ea{r}_{apair}")
                c, h = r // 2, r % 2
                for up in range(2):
                    nc.vector.tensor_copy(
                        ea[up * 64:(up + 1) * 64],
                        esb[c][h * 64:(h + 1) * 64, :, up])
                u1, u2 = r // 2, r % 2
                for m in range(2):
                    psf = pp_f.tile([128, 2 * 192], f32, tag="f")
                    for g in range(2):
                        nc.tensor.matmul(
                            psf[:], sb["projt"][:, g, m, :],
                            ea[:, :, g, :, :, :],
                            start=(g == 0), stop=(g == 1))
                    src = psf[:].rearrange(
                        "c (a p side q bl) -> c a p side q bl",
                        a=2, p=2, side=2, q=2)
                    dst = stage[m][:, u1, :, :, :, :, :, u2]
                    if not (m == 1 and r % 2 == 1):
                        nc.scalar.activation(dst, src, AF.Relu,
                                             bias=sb["bias2"][:, m, :])
                    else:
                        nc.vector.tensor_scalar(
                            dst, src, sb["bias2"][:, m, :], 0.0,
                            op0=ALU.add, op1=ALU.max)
                    if r % 2 == 1:
                        # u1 slice complete -> DMA it out. u1 is the
                        # outermost stage dim, so later r-writes land
                        # in a disjoint byte range (no false WAR).
                        # DRAM layout [m][c][pair][u1][a2][p][col]
                        # (bf16, unshuffled on host) keeps descriptors
                        # contiguous.
                        u1x = r // 2
                        base2 = apair * 1536 + u1x * 768
                        dsta = bass.AP(d_out[m].tensor,
                                       d_out[m].offset + base2,
                                       [d_out[m].ap[0], [1, 768]])
                        nc.sync.dma_start(
                            dsta,
                            stage[m][:, u1x].rearrange(
                                "c a p side q bl v -> c (a p side q bl v)"))


def _build_program():
    import concourse.bacc as bacc
    import concourse.tile as tile
    from concourse import mybir

    dts = {"bf16": mybir.dt.bfloat16, "f32": mybir.dt.float32,
           "i16": mybir.dt.int16}
    nc = bacc.Bacc("TRN2", target_bir_lowering=False, debug=False,
                   num_devices=NCORES)
    ins = {}
    for name, shape, dt in INPUT_SPECS:
        ins[name] = nc.dram_tensor(name, shape, dts[dt],
                                   kind="ExternalInput").ap()
    outs = {"out": nc.dram_tensor("out", [2, 128, 48 * FW],
                                  mybir.dt.bfloat16,
                                  kind="ExternalOutput").ap()}
    with tile.TileContext(nc) as tc:
        device_kernel(tc, outs, ins)
    nc.compile()
    return nc


def _get_program():
    if "nc" not in _CACHE:
        _CACHE["nc"] = _build_program()
    return _CACHE["nc"]


def _core_in_map(hp, core):
    import ml_dtypes
    bf16 = ml_dtypes.bfloat16
    cs = core_slices(hp, core)

    w1t = np.stack([hp["W1"][:, :128].T, hp["W1"][:, 128:].T],
                   axis=1)                            # [128, 2, 64]
    ke1t = np.zeros((Cc, 9, Cc), dtype=np.float32)
    ke2t = np.zeros((Cc, 9, CK), dtype=np.float32)
    for ky in range(3):
        for kx in range(3):
            t = ky * 3 + kx
            ke1t[:, t, :] = hp["W2"][:, :, ky, kx].T
            ke2t[:, t, :] = hp["W3"][:, :, ky, kx].T
    cm2 = hp["cm2"].transpose(2, 0, 1, 3).reshape(CK, 10, 72)
    # contract-128 proj weights: row up*64+m <-> cin = (2g+up)*64 + m
    projt = np.zeros((128, 2, 2, 128), dtype=np.float32)
    for g in range(2):
        for up in range(2):
            u = 2 * g + up
            for mh in range(2):
                projt[up * 64:(up + 1) * 64, g, mh, :] = \
                    hp["P"][mh * 128:(mh + 1) * 128,
                            u * 64:(u + 1) * 64].T
    gden = np.zeros((CK, 4), dtype=np.float32)
    for u in range(4):
        gden[u * 25:(u + 1) * 25, u] = 1.0
    g2 = np.ascontiguousarray(gden.T)
    bias2 = np.zeros((128, 2, 1), dtype=np.float32)
    bias2[:, 0, 0] = hp["bias2"][:128]
    bias2[:, 1, 0] = hp["bias2"][128:]
    qa = np.zeros((CK, 2), dtype=np.float32)
    qa[:, 0] = 0.25 * (1.0 - cs["e_top"])
    qa[:, 1] = 0.75 + 0.25 * cs["e_top"]
    ebot = np.zeros((CK, 2), dtype=np.float32)
    ebot[:, 0] = cs["e_bot"]
    ebot[:, 1] = 1.0 - cs["e_bot"]
    mc = np.concatenate([cs["mask_comp"][0:4], cs["mask_comp"][16:20]])
    mk = np.concatenate([cs["mask_k1"][0:2], cs["mask_k1"][14:16]])
    mask1 = np.broadcast_to(mc[None, :, None], (Cc, 8, 52)).copy()
    mask2 = np.broadcast_to(mk[None, :, None], (Cc, 4, 50)).copy()

    return {
        "xin": np.ascontiguousarray(cs["xin"].transpose(1, 0, 2, 3)).astype(bf16),
        "w1t": w1t.astype(bf16),
        "ke1t": ke1t.astype(bf16),
        "ke2t": ke2t.astype(bf16),
        "cm2": np.ascontiguousarray(cm2).astype(bf16),
        "projt": projt.astype(bf16),
        "gden": gden.astype(bf16),
        "g2": g2.astype(bf16),
        "srep": hp["srep"].astype(bf16),
        "sidx": hp["sidx"],
        "xt3": cs["xt3"].astype(bf16),
        "mask1": mask1.astype(bf16),
        "mask2": mask2.astype(bf16),
        "bias1": np.ascontiguousarray(hp["bias1"][:, None]),
        "biask": np.ascontiguousarray(hp["biask"][:, None]),
        "bias2": bias2,
        "qa": qa,
        "ebot": ebot,
    }


def _gather(results):
    out = np.zeros((N, C, FH, FW), dtype=np.float32)
    for core in range(NCORES):
        n, blk = core // 4, core % 4
        a0 = blk * BLK
        # device layout [m][c][pair][u1][a][p][side][q][bl][u2]:
        # row = (pair, a, p, u1), col = 96*side + 4*bl + 2*q + u2
        o = results[core]["out"].astype(np.float32).reshape(
            2, 128, 6, 2, 2, 2, 2, 2, 24, 2)
        o = o.transpose(0, 1, 2, 4, 5, 3, 6, 8, 7, 9).reshape(
            2, 128, 48, FW)
        out[n, :128, 4 * a0: 4 * a0 + 48, :] = o[0]
        out[n, 128:, 4 * a0: 4 * a0 + 48, :] = o[1]
    return out


def kernel(**inputs) -> np.ndarray:
    if os.environ.get("CARAFE_EMULATE"):
        return emulate(inputs)
    from concourse.bass_utils import run_bass_kernel_spmd
    hp = host_prep(inputs)
    nc = _get_program()
    in_maps = [_core_in_map(hp, core) for core in range(NCORES)]
    res = run_bass_kernel_spmd(nc, in_maps, list(range(NCORES)),
                               trace=bool(os.environ.get("CARAFE_TRACE")))
    _CACHE["last_results"] = res
    return _gather(res.results)



# revision 81
# speedup vs baseline: 1.0121x; 1.0121x over previous
"""
ContentAwareUpsampling (CARAFE-style) Trainium2 Bass kernel.

x[2,256,48,48] -> out[2,256,192,192]; 8 cores = 2 batches x 4 blocks of 12 rows.

Decomposition (validated bit-close by emulate() against the jax reference):
  branch convs (bf16 matmuls, BN+relu fused into ACT drains)
  softmax via exp (ACT) + group-sum/broadcast matmuls + reciprocal
  T = collapse(bilinear(softmax)) as 9 shifted matmuls vs constant [100,144]
  band matrices B built by GPSIMD local_scatter with constant indices
  einsum  E[c,(u,q,b)] = sum_dy xT[row a+dy].T @ B[dy]     (per row a, parity p)
  proj    F[r] = sum_u P_sub[u].T @ E_sigma[r*64:+64]      (sigma baked in xt)
  pixel-shuffle + BN + relu fused into the ACT drains; DMA out.
"""

import os
import numpy as np

# ---------------------------------------------------------------- constants
N, C, H, W = 2, 256, 48, 48
Cc, CK, UP, K5 = 64, 100, 2, 5
EPS = 1e-5
NCORES = 8
BLK = 12            # lo-res rows per core block
R_XIN = 20          # x rows for branch (a0-4 .. a0+15)
R_XT = 14           # x rows for einsum (a0-1 .. a0+12)
R_KN = 14           # kernel rows (a0-1 .. a0+12)
OH, OW = H * UP, W * UP          # 96
FH, FW = OH * UP, OW * UP        # 192

DYMAP = {0: [-1, -1, 0, 0, 1], 1: [-1, 0, 0, 1, 1]}
BLEND = {0: {-1: 0.25, 0: 0.75}, 1: {0: 0.75, 1: 0.25}}


def _f32(a):
    return np.ascontiguousarray(a, dtype=np.float32)


# ---------------------------------------------------------------- host prep
def _fold_bn(w, g, b, m, v):
    inv = g / np.sqrt(v + EPS)
    return w * inv[:, None], b - m * inv


def build_c2mats():
    """C2[(rho+1)*5 + (s+2)] [100, 144]: D2[b', (p,dy,q,u,dx)] =
    sum_{rho,s} Kn[:, j+rho+1, b'+s+2].T @ C2, with tau = s + dx."""
    cm = np.zeros((15, CK, 144), dtype=np.float32)
    for p in range(2):
        for q in range(2):
            for u in range(4):
                for ky in range(K5):
                    dy = DYMAP[p][ky]
                    for kx in range(K5):
                        dx = DYMAP[q][kx]
                        col = p * 72 + (dy + 1) * 24 + q * 12 + u * 3 + (dx + 1)
                        row = u * 25 + ky * 5 + kx
                        for rho, br in BLEND[p].items():
                            for tau, bc in BLEND[q].items():
                                s = tau - dx
                                w = (rho + 1) * 5 + (s + 2)
                                cm[w, row, col] += br * bc
    return cm


def build_cm2():
    """cm2[p, s'] [100, 72]: D2_p[b', (dy,q,u,dx)] =
    sum_{s'} Kb_p[:, j, b'+s'].T @ cm2[p, s'], where Kb_p holds the
    rho-blended kernel rows (blend folded on DVE) and tau = s' - 2 + dx."""
    cm = np.zeros((2, 5, CK, 72), dtype=np.float32)
    for p in range(2):
        for q in range(2):
            for u in range(4):
                for ky in range(K5):
                    dy = DYMAP[p][ky]
                    for kx in range(K5):
                        dx = DYMAP[q][kx]
                        col = (dy + 1) * 24 + q * 12 + u * 3 + (dx + 1)
                        row = u * 25 + ky * 5 + kx
                        for tau, bc in BLEND[q].items():
                            s = tau - dx
                            cm[p, s + 2, row, col] += bc
    return cm


# einsum-u -> B column block. Block k = up*2 + g where u = 2g + up, so the
# PSUM einsum output comes out ordered (up, g, q, b) and the proj repack can
# slice by u-parity with plain partition-offset copies.
U2BLK = [0, 2, 1, 3]


def build_scat_idx():
    """Scatter index set for one [80, 192] B half-tile per (j, p, side).
    Partition rho = dy*26 + w holds x column (cbase + w - 1); its source
    data is the dy-replicated D2 p-block [80, 72] with cols (dy', q, u,
    dx'); entries with dy' != dy (or out-of-window b) are -1 (ignored).
    Output cols are (ublk4, q2, bl24) with bl = w - 1 - dx."""
    sidx = np.full((80, 72), -1, dtype=np.int16)
    for dy in range(3):
        for w in range(26):
            rho = dy * 26 + w
            for q in range(2):
                for u in range(4):
                    for dx_i in range(3):
                        t = dy * 24 + q * 12 + u * 3 + dx_i
                        bl = w - dx_i
                        if 0 <= bl < 24:
                            sidx[rho, t] = U2BLK[u] * 48 + q * 24 + bl
    return sidx


def host_prep(inputs):
    x = _f32(inputs["x"])
    W1, bias1 = _fold_bn(_f32(inputs["compress_w"])[:, :, 0, 0],
                         _f32(inputs["g1"]), _f32(inputs["b1"]),
                         _f32(inputs["m1"]), _f32(inputs["v1"]))
    invk = _f32(inputs["gk"]) / np.sqrt(_f32(inputs["vk"]) + EPS)
    W2 = _f32(inputs["ke1_w"]) * invk[:, None, None, None]
    biask = _f32(inputs["bk"]) - _f32(inputs["mk"]) * invk
    W3 = _f32(inputs["ke2_w"])
    P, bias2 = _fold_bn(_f32(inputs["proj_w"])[:, :, 0, 0],
                        _f32(inputs["g2"]), _f32(inputs["b2"]),
                        _f32(inputs["m2"]), _f32(inputs["v2"]))
    cmats = build_c2mats()
    cm2 = build_cm2()
    sidx = build_scat_idx()
    # dy-replication selector: D2rep[dy*26+w', :] = D2[w' + 24*side, :]
    srep = np.zeros((50, 2, 78), dtype=np.float32)
    for side in range(2):
        for dy in range(3):
            for w in range(26):
                srep[w + 24 * side, side, dy * 26 + w] = 1.0
    # sigma position p = r*64 + m  ->  channel c = 4m + r
    sigma = np.array([4 * (p % 64) + p // 64 for p in range(256)])
    return dict(x=x, W1=W1, bias1=bias1, W2=W2, biask=biask, W3=W3,
                P=P, bias2=bias2, cmats=cmats, cm2=cm2, sidx=sidx,
                srep=srep, sigma=sigma)


def core_slices(hp, core):
    n, blk = core // 4, core % 4
    a0 = blk * BLK
    x = hp["x"][n]

    xin = np.zeros((2, 128, R_XIN, 52), dtype=np.float32)
    for s in range(R_XIN):
        row = a0 - 4 + s
        if 0 <= row < H:
            xin[0, :, s, 2:50] = x[:128, row, :]
            xin[1, :, s, 2:50] = x[128:, row, :]

    # xt kept for the numpy emulation path only
    xs = x[hp["sigma"]]
    xt = np.zeros((50, R_XT, 2, 128), dtype=np.float32)
    for s in range(R_XT):
        row = a0 - 1 + s
        if 0 <= row < H:
            xt[1:49, s, 0, :] = xs[:128, row, :].T
            xt[1:49, s, 1, :] = xs[128:, row, :].T

    # xt3: dy-replicated band windows. Partition dy*26 + w holds x column
    # (cbase(side) + w - 1) of lo-res row a0 + j - 1 + dy (0 if OOB).
    xt3 = np.zeros((80, BLK, 2, 2, 128), dtype=np.float32)
    for dy in range(3):
        for w in range(26):
            rho = dy * 26 + w
            for j in range(BLK):
                row = a0 + j - 1 + dy
                if not (0 <= row < H):
                    continue
                for side in range(2):
                    xcol = (side * 24) + w - 1
                    if 0 <= xcol < W:
                        xt3[rho, j, side, 0, :] = xs[:128, row, xcol]
                        xt3[rho, j, side, 1, :] = xs[128:, row, xcol]

    e_top = 1.0 if a0 == 0 else 0.0
    e_bot = 1.0 if a0 + BLK == H else 0.0
    mask_comp = np.array([1.0 if 0 <= a0 - 4 + s < H else 0.0
                          for s in range(R_XIN)], dtype=np.float32)
    mask_k1 = np.array([1.0 if 0 <= a0 - 2 + s < H else 0.0
                        for s in range(16)], dtype=np.float32)
    return dict(a0=a0, n=n, xin=xin, xt=xt, xt3=xt3, e_top=e_top,
                e_bot=e_bot, mask_comp=mask_comp, mask_k1=mask_k1)


# ---------------------------------------------------------------- emulation
def emulate_core(hp, cs):
    xin, xt = cs["xin"], cs["xt"]
    W1, W2, W3 = hp["W1"], hp["W2"], hp["W3"]

    xin_flat = np.concatenate([xin[0], xin[1]], axis=0)
    comp = np.zeros((Cc, R_XIN, 52), dtype=np.float32)
    comp[:, :, 2:50] = np.maximum(
        np.einsum("oc,csw->osw", W1, xin_flat[:, :, 2:50])
        + hp["bias1"][:, None, None], 0.0)
    comp *= cs["mask_comp"][None, :, None]

    k1 = np.zeros((Cc, 16, 50), dtype=np.float32)
    acc = np.zeros((Cc, 16, 48), dtype=np.float32)
    for ky in range(3):
        for kx in range(3):
            sh = comp[:, ky * 2: ky * 2 + 16, kx * 2: kx * 2 + 48]
            acc += np.einsum("oi,isw->osw", W2[:, :, ky, kx], sh)
    k1[:, :, 1:49] = np.maximum(acc + hp["biask"][:, None, None], 0.0)
    k1 *= cs["mask_k1"][None, :, None]

    kc = np.zeros((CK, R_KN, 48), dtype=np.float32)
    for ky in range(3):
        for kx in range(3):
            kc += np.einsum("oi,isw->osw", W3[:, :, ky, kx],
                            k1[:, ky: ky + 14, kx: kx + 48])

    e = np.exp(kc.reshape(4, 25, R_KN, 48))
    kn_int = (e / e.sum(axis=1, keepdims=True)).reshape(CK, R_KN, 48)

    # Kn [100, 14, 54]: col i <-> kernel pixel b = i - 3; interior i 3..50
    Kn = np.zeros((CK, R_KN, 54), dtype=np.float32)
    Kn[:, :, 3:51] = kn_int
    Kn[:, :, 2] = Kn[:, :, 3]
    Kn[:, :, 51] = Kn[:, :, 50]
    if cs["e_top"] == 1.0:
        Kn[:, 0, :] = Kn[:, 1, :]
    if cs["e_bot"] == 1.0:
        Kn[:, 13, :] = Kn[:, 12, :]

    cmats, P = hp["cmats"], hp["P"]
    scat_idx = np.full((64, 72), -1, dtype=np.int16)
    for bp in range(50):
        for dy_i in range(3):
            for q in range(2):
                for u in range(4):
                    for dx_i in range(3):
                        t = dy_i * 24 + q * 12 + u * 3 + dx_i
                        b = bp - 1 - (dx_i - 1)
                        if 0 <= b < 48:
                            scat_idx[bp, t] = dy_i * 384 + u * 96 + q * 48 + b
    out = np.zeros((C, 4 * BLK, FW), dtype=np.float32)
    for j in range(BLK):
        # D2 [50, (p, dy, q, u, dx)]
        D2 = np.zeros((50, 144), dtype=np.float32)
        for rho in (-1, 0, 1):
            for s in (-2, -1, 0, 1, 2):
                w = (rho + 1) * 5 + (s + 2)
                D2 += Kn[:, j + rho + 1, s + 2: s + 52].T @ cmats[w]
        for p in range(2):
            D = np.zeros((64, 72), dtype=np.float32)
            D[0:50] = D2[:, p * 72:(p + 1) * 72]
            B = np.zeros((64, 1152), dtype=np.float32)
            for bp in range(64):
                for t in range(72):
                    ix = scat_idx[bp, t]
                    if ix >= 0:
                        B[bp, ix] = D[bp, t]
            E = np.zeros((256, 384), dtype=np.float32)
            for ch in range(2):
                acc = np.zeros((128, 384), dtype=np.float32)
                for dy_i in range(3):
                    xsl = xt[0:50, j + dy_i, ch, :]
                    acc += xsl.T @ B[0:50, dy_i * 384:(dy_i + 1) * 384]
                E[ch * 128:(ch + 1) * 128] = acc
            for r in range(4):
                u1, u2 = r // 2, r % 2
                F = np.zeros((C, 96), dtype=np.float32)
                for u in range(4):
                    F += P[:, u * 64:(u + 1) * 64] @ \
                        E[r * 64:(r + 1) * 64, u * 96:(u + 1) * 96]
                F = np.maximum(F + hp["bias2"][:, None], 0.0)
                Y = 4 * j + 2 * p + u1
                Fq = F.reshape(C, 2, 48)
                for q in range(2):
                    out[:, Y, u2 + 2 * q::4] = Fq[:, q, :]
    return out


def emulate(inputs):
    hp = host_prep(inputs)
    out = np.zeros((N, C, FH, FW), dtype=np.float32)
    for core in range(NCORES):
        cs = core_slices(hp, core)
        n, a0 = cs["n"], cs["a0"]
        out[n, :, 4 * a0: 4 * a0 + 48, :] = emulate_core(hp, cs)
    return out


# ---------------------------------------------------------------- device
_CACHE = {}

INPUT_SPECS = [
    # DMA issue order == list order; w1t first so the PE's ldweights can
    # start the p-state ramp while xin is still in flight
    ("w1t", [128, 2, Cc], "bf16"),
    ("bias1", [Cc, 1], "f32"),
    ("xin", [128, 2, R_XIN, 52], "bf16"),
    ("mask1", [Cc, 8, 52], "bf16"),
    ("ke1t", [Cc, 9, Cc], "bf16"),
    ("biask", [Cc, 1], "f32"),
    ("mask2", [Cc, 4, 50], "bf16"),
    ("ke2t", [Cc, 9, CK], "bf16"),
    ("gden", [CK, 4], "bf16"),
    ("g2", [4, CK], "bf16"),
    ("qa", [CK, 2], "f32"),
    ("ebot", [CK, 2], "f32"),
    ("cm2", [CK, 10, 72], "bf16"),
    ("srep", [50, 2, 78], "bf16"),
    ("sidx", [80, 72], "i16"),
    ("xt3", [80, BLK, 2, 2, 128], "bf16"),
    ("bias2", [128, 2, 1], "f32"),
    ("projt", [128, 2, 2, 128], "bf16"),
]


def device_kernel(tc, outs, ins):
    """Emit the kernel. outs/ins: dicts name -> bass.AP (DRAM)."""
    from contextlib import ExitStack
    import concourse.bass as bass
    from concourse import mybir

    f32 = mybir.dt.float32
    bf16 = mybir.dt.bfloat16
    i16 = mybir.dt.int16
    AF = mybir.ActivationFunctionType
    ALU = mybir.AluOpType
    nc = tc.nc
    d_out = outs["out"]

    with ExitStack() as ctx:
        sing = ctx.enter_context(tc.tile_pool(name="sing", bufs=1))

        # ---- load inputs/constants (critical-path first, bulky ones later)
        sb = {}
        dts = {"bf16": bf16, "f32": f32, "i16": i16}
        for name, shape, dt in INPUT_SPECS:
            t = sing.tile(shape[:], dts[dt], tag=name)
            sb[name] = t
            if name == "xt3":
                nc.sync.dma_start(t[:, 0:2], ins[name][:, 0:2])
                nc.sync.dma_start(t[:, 2:6], ins[name][:, 2:6])
                nc.sync.dma_start(t[:, 6:12], ins[name][:, 6:12])
            elif name == "xin":
                # split by c-half: conv1's first matmul only needs c=0
                nc.sync.dma_start(t[:, 0], ins[name][:, 0])
                nc.sync.dma_start(t[:, 1], ins[name][:, 1])
            else:
                nc.sync.dma_start(t[:], ins[name])

        # persistent activations. All memsets go to the (otherwise idle at
        # start) GPSIMD engine, and only cover the pad regions the compute
        # never writes.
        dsbs = [sing.tile([80, 144], bf16, name=f"dsbr{i}") for i in range(4)]
        for t in dsbs:
            nc.gpsimd.memset(t[:], 0.0)
        dsb50s = [sing.tile([50, 144], bf16, name=f"dsb50r{i}")
                  for i in range(4)]
        comp = sing.tile([Cc, R_XIN, 52], bf16)
        k1 = sing.tile([Cc, 16, 50], bf16)
        exps = sing.tile([CK, R_KN, 48], bf16)
        recs = sing.tile([4, R_KN, 48], bf16)
        kn = sing.tile([CK, R_KN, 54], bf16)
        kb = sing.tile([CK, 2, BLK, 54], bf16)
        t75 = sing.tile([CK, BLK, 54], bf16)
        tb = sing.tile([CK, 54], bf16)
        nc.gpsimd.memset(comp[:, :, 0:2], 0.0)
        nc.gpsimd.memset(comp[:, :, 50:52], 0.0)
        nc.gpsimd.memset(k1[:, :, 0:1], 0.0)
        nc.gpsimd.memset(k1[:, :, 49:50], 0.0)
        nc.gpsimd.memset(kn[:, :, 0:3], 0.0)
        nc.gpsimd.memset(kn[:, :, 51:54], 0.0)

        def emit_kb(jr):
            # rho-blend fold: kb[p=0,j] = .25 kn[j] + .75 kn[j+1]
            #                 kb[p=1,j] = .75 kn[j+1] + .25 kn[j+2]
            lo, hi = jr.start, jr.stop
            nc.vector.tensor_scalar_mul(t75[:, jr, :],
                                        kn[:, lo + 1:hi + 1, :], 0.75)
            nc.vector.scalar_tensor_tensor(
                kb[:, 0, jr, :], kn[:, lo:hi, :], 0.25,
                t75[:, jr, :], op0=ALU.mult, op1=ALU.add)
            nc.vector.scalar_tensor_tensor(
                kb[:, 1, jr, :], kn[:, lo + 2:hi + 2, :], 0.25,
                t75[:, jr, :], op0=ALU.mult, op1=ALU.add)

        # pools used by emit_tdb (created early; 1 PSUM bank + branch's 6)
        bpool = ctx.enter_context(tc.tile_pool(name="bb", bufs=6))
        pp_t = ctx.enter_context(tc.tile_pool(name="pp_t", bufs=1, space="PSUM"))
        bts = {}

        def emit_t(j):
            # T collapse for row j: D2 [50, 144] -> bf16 dsb50 ring
            pst = pp_t.tile([50, 144], f32, tag="t", name=f"pst{j}")
            for p in range(2):
                for s in range(5):
                    nc.tensor.matmul(
                        pst[:, p * 72:(p + 1) * 72], kb[:, p, j, s: s + 50],
                        sb["cm2"][:, p * 5 + s, :],
                        start=(s == 0), stop=(s == 4))
            nc.scalar.copy(dsb50s[j % 4][:], pst[:])

        def emit_b(j):
            # dy-replication via a constant 0/1 selection matmul (the
            # matmul weights AP must be 1-D, so a [dy3, w26] kb slice is
            # not allowed), then one band scatter per (p, side)
            psr = pp_t.tile([78, 2, 144], f32, tag="r", name=f"psr{j}")
            for side in range(2):
                nc.tensor.matmul(psr[:, side, :], sb["srep"][:, side, :],
                                 dsb50s[j % 4][:], start=True, stop=True)
                dsb = dsbs[(2 * j + side) % 4]
                if side == 0:
                    nc.scalar.copy(dsb[0:78, :], psr[:, side, :])
                else:
                    nc.vector.tensor_copy(dsb[0:78, :], psr[:, side, :])
                for p in range(2):
                    bt = bpool.tile([80, 192], bf16, tag="b",
                                    name=f"bt{j}_{p}_{side}")
                    nc.gpsimd.local_scatter(
                        bt[:], dsb[:, p * 72:(p + 1) * 72],
                        sb["sidx"][:], channels=80,
                        num_elems=192, num_idxs=72)
                    bts[(j, p, side)] = bt

        def emit_tdb(j):
            emit_t(j)
            emit_b(j)

        # ---- branch phase
        # chunked so early rows flow to the main loop ASAP:
        #   comp rows 0:7 | 7:14 | 14:20 ; k1 rows 0:10 | 10:16 ;
        #   kn rows 0:8 | 8:14
        with tc.tile_pool(name="brp", bufs=2, space="PSUM") as brp:
            for h, (c0, CL) in enumerate(((0, 7), (7, 7), (14, 6))):
                ps = brp.tile([Cc, CL * 48], f32, tag="psc")
                for c in range(2):
                    nc.tensor.matmul(
                        ps[:], sb["w1t"][:, c, :],
                        sb["xin"][:, c, c0:c0 + CL, 2:50],
                        start=(c == 0), stop=(c == 1))
                rs = slice(c0, c0 + CL)
                if h == 0:
                    nc.vector.tensor_scalar(
                        comp[:, rs, 2:50],
                        ps[:].rearrange("c (r w) -> c r w", w=48),
                        sb["bias1"][:], 0.0, op0=ALU.add, op1=ALU.max)
                else:
                    nc.scalar.activation(
                        comp[:, rs, 2:50],
                        ps[:].rearrange("c (r w) -> c r w", w=48),
                        AF.Relu, bias=sb["bias1"][:])
                # only rows 0:4 (top core) / 16:20 (bottom core) can be OOB
                if h == 0:
                    nc.vector.tensor_mul(comp[:, 0:4, :], comp[:, 0:4, :],
                                         sb["mask1"][:, 0:4, :])
                elif h == 2:
                    nc.vector.tensor_mul(comp[:, 16:20, :],
                                         comp[:, 16:20, :],
                                         sb["mask1"][:, 4:8, :])
            for r0, L in ((0, 10), (10, 6)):
                ps = brp.tile([Cc, L * 48], f32, tag="psk1")
                for ky in range(3):
                    for kx in range(3):
                        t = ky * 3 + kx
                        nc.tensor.matmul(
                            ps[:], sb["ke1t"][:, t, :],
                            comp[:, r0 + ky * 2: r0 + ky * 2 + L,
                                 kx * 2: kx * 2 + 48],
                            start=(t == 0), stop=(t == 8))
                rs = slice(r0, r0 + L)
                nc.vector.tensor_scalar(
                    k1[:, rs, 1:49],
                    ps[:].rearrange("c (r w) -> c r w", w=48),
                    sb["biask"][:], 0.0, op0=ALU.add, op1=ALU.max)
                # only rows 0:2 (top core) / 14:16 (bottom core) can be OOB
                mr = slice(0, 2) if r0 == 0 else slice(14, 16)
                ms = slice(0, 2) if r0 == 0 else slice(2, 4)
                nc.vector.tensor_mul(k1[:, mr, :], k1[:, mr, :],
                                     sb["mask2"][:, ms, :])
            for ci, (r0, L) in enumerate(((0, 8), (8, 6))):
                rs = slice(r0, r0 + L)
                ps = brp.tile([CK, L * 48], f32, tag="psk2", bufs=2)
                for ky in range(3):
                    for kx in range(3):
                        t = ky * 3 + kx
                        nc.tensor.matmul(
                            ps[:], sb["ke2t"][:, t, :],
                            k1[:, r0 + ky: r0 + ky + L, kx: kx + 48],
                            start=(t == 0), stop=(t == 8))
                nc.scalar.activation(
                    exps[:, rs, :],
                    ps[:].rearrange("c (r w) -> c r w", w=48), AF.Exp)
                psd = brp.tile([4, L * 48], f32, tag="psc")
                nc.tensor.matmul(psd[:], sb["gden"][:], exps[:, rs, :],
                                 start=True, stop=True)
                with nc.allow_low_precision(reason="bf16 softmax pipeline"):
                    nc.vector.reciprocal(
                        recs[:, rs, :],
                        psd[:].rearrange("c (r w) -> c r w", w=48))
                psb = brp.tile([CK, L * 48], f32, tag="psc")
                nc.tensor.matmul(psb[:], sb["g2"][:], recs[:, rs, :],
                                 start=True, stop=True)
                nc.vector.tensor_mul(
                    kn[:, rs, 3:51], exps[:, rs, :],
                    psb[:].rearrange("c (r w) -> c r w", w=48))
                nc.vector.tensor_copy(kn[:, rs, 2:3], kn[:, rs, 3:4])
                nc.vector.tensor_copy(kn[:, rs, 51:52], kn[:, rs, 50:51])
                if ci == 0:
                    # kb rows 0/1 with the top-edge row blend folded into
                    # row 0's coefficients: kb[0,0] = a*kn0 + b*kn1 where
                    # a = .25*(1-e_top), b = .75 + .25*e_top
                    nc.scalar.mul(t75[:, 0:2, :], kn[:, 1:3, :], 0.75)
                    nc.scalar.mul(tb[:], kn[:, 1, :], sb["qa"][:, 1:2])
                    nc.vector.scalar_tensor_tensor(
                        kb[:, 0, 0, :], kn[:, 0, :], sb["qa"][:, 0:1],
                        tb[:], op0=ALU.mult, op1=ALU.add)
                    nc.vector.scalar_tensor_tensor(
                        kb[:, 0, 1, :], kn[:, 1, :], 0.25,
                        t75[:, 1, :], op0=ALU.mult, op1=ALU.add)
                    nc.vector.scalar_tensor_tensor(
                        kb[:, 1, 0:2, :], kn[:, 2:4, :], 0.25,
                        t75[:, 0:2, :], op0=ALU.mult, op1=ALU.add)
                    emit_kb(slice(1, 6))
                    # start the T chains for the first rows while the
                    # rest of the branch still runs
                    emit_t(0)
                    emit_t(1)
                    emit_t(2)
                    emit_t(3)
                    emit_b(0)
                    emit_b(1)
                else:
                    nc.vector.tensor_scalar_mul(kn[:, 13, :], kn[:, 13, :],
                                                sb["ebot"][:, 1:2])
                    nc.vector.scalar_tensor_tensor(
                        kn[:, 13, :], kn[:, 12, :], sb["ebot"][:, 0:1],
                        kn[:, 13, :], op0=ALU.mult, op1=ALU.add)

        # ---- main loop
        epool = ctx.enter_context(tc.tile_pool(name="esb", bufs=3))
        eapool = ctx.enter_context(tc.tile_pool(name="ea", bufs=2))
        spool = ctx.enter_context(tc.tile_pool(name="stage", bufs=3))
        pp_e = ctx.enter_context(tc.tile_pool(name="pp_e", bufs=2, space="PSUM"))
        pp_f = ctx.enter_context(tc.tile_pool(name="pp_f", bufs=2, space="PSUM"))

        esb = {}
        stage = {}

        emit_kb(slice(6, 12))
        for apair in range(BLK // 2):
            j0 = 2 * apair
            # esb: [c-half, a2, up2, g2, p2, q2, b48]
            # esb: [c-half, a2, up2, g2, p2, side2, q2, bl24]
            esb = {c: epool.tile([128, 2, 2, 2, 2, 2, 2, 24], bf16,
                                 tag=f"e{c}", name=f"esb{c}")
                   for c in range(2)}
            # [c, u1, a2, p, b, q, u2]: u1 outermost so each DMA
            # slice is one contiguous 768-element run per partition
            # stage: [c, u1, a2, p2, side2, q2, bl24, u2]
            stage = {m: spool.tile([128, 2, 2, 2, 2, 2, 24, 2], bf16,
                                   tag=f"s{m}", name=f"stage{m}")
                     for m in range(2)}
            for j in (j0, j0 + 1):
                btp = {(p, side): bts.pop((j, p, side))
                       for p in range(2) for side in range(2)}
                for c in range(2):
                    # one PSUM tile covers both parities -> one wide drain.
                    # Each (p, side) matmul writes a contiguous 192-col
                    # region (matmul PSUM outputs must not be strided);
                    # the drain's AP permutation restores esb's layout.
                    pse = pp_e.tile([128, 2, 512], f32, tag="e")
                    for p in range(2):
                        for side in range(2):
                            nc.tensor.matmul(
                                pse[:, p, side * 192:side * 192 + 192],
                                sb["xt3"][0:78, j, side, c, :],
                                btp[(p, side)][0:78, :],
                                start=True, stop=True)
                    if c == 0:
                        # DVE's table-driven lowering accepts the 4-D
                        # pattern; one wide drain
                        nc.vector.tensor_copy(
                            esb[c][:, j % 2],
                            pse[:, :, 0:384].rearrange(
                                "c p (side up g q bl) -> c up g p side q bl",
                                side=2, up=2, g=2, q=2))
                    else:
                        # ACT codegen is limited to 3-D free patterns;
                        # split by parity
                        for p in range(2):
                            nc.scalar.copy(
                                esb[c][:, j % 2, :, :, p],
                                pse[:, p, 0:384].rearrange(
                                    "c (side up g q bl) -> c up g side q bl",
                                    side=2, up=2, g=2, q=2))
                # staggered T/B prefetch: B for j+2 (whose T ran last
                # apair, so its drain has landed), T for j+4. Keeps
                # independent PE runway between einsum and proj.
                if j + 2 < BLK:
                    emit_b(j + 2)
                if j + 4 < BLK:
                    emit_t(j + 4)

            # repack + proj, r-interleaved: proj(r) starts as soon as its
            # own two repack copies land. eA[r] rows up*64+m hold channel
            # (4m+r) at einsum-u = 2g+up (g in the free dim); the copies
            # are plain partition-offset moves that DVE runs in 4x mode.
            for r in range(4):
                ea = eapool.tile([128, 2, 2, 2, 2, 2, 24], bf16,
                                 tag=f"ea{r}", name=f"ea{r}_{apair}")
                c, h = r // 2, r % 2
                for up in range(2):
                    nc.vector.tensor_copy(
                        ea[up * 64:(up + 1) * 64],
                        esb[c][h * 64:(h + 1) * 64, :, up])
                u1, u2 = r // 2, r % 2
                for m in range(2):
                    psf = pp_f.tile([128, 2 * 192], f32, tag="f")
                    for g in range(2):
                        nc.tensor.matmul(
                            psf[:], sb["projt"][:, g, m, :],
                            ea[:, :, g, :, :, :],
                            start=(g == 0), stop=(g == 1))
                    src = psf[:].rearrange(
                        "c (a p side q bl) -> c a p side q bl",
                        a=2, p=2, side=2, q=2)
                    dst = stage[m][:, u1, :, :, :, :, :, u2]
                    if not (m == 1 and r % 2 == 1):
                        nc.scalar.activation(dst, src, AF.Relu,
                                             bias=sb["bias2"][:, m, :])
                    else:
                        nc.vector.tensor_scalar(
                            dst, src, sb["bias2"][:, m, :], 0.0,
                            op0=ALU.add, op1=ALU.max)
                    if r % 2 == 1:
                        # u1 slice complete -> DMA it out. u1 is the
                        # outermost stage dim, so later r-writes land
                        # in a disjoint byte range (no false WAR).
                        # DRAM layout [m][c][pair][u1][a2][p][col]
                        # (bf16, unshuffled on host) keeps descriptors
                        # contiguous.
                        u1x = r // 2
                        base2 = apair * 1536 + u1x * 768
                        dsta = bass.AP(d_out[m].tensor,
                                       d_out[m].offset + base2,
                                       [d_out[m].ap[0], [1, 768]])
                        nc.sync.dma_start(
                            dsta,
                            stage[m][:, u1x].rearrange(
                                "c a p side q bl v -> c (a p side q bl v)"))


def _build_program():
    import concourse.bacc as bacc
    import concourse.tile as tile
    from concourse import mybir

    dts = {"bf16": mybir.dt.bfloat16, "f32": mybir.dt.float32,
           "i16": mybir.dt.int16}
    nc = bacc.Bacc("TRN2", target_bir_lowering=False, debug=False,
                   num_devices=NCORES)
    ins = {}
    for name, shape, dt in INPUT_SPECS:
        ins[name] = nc.dram_tensor(name, shape, dts[dt],
                                   kind="ExternalInput").ap()
    outs = {"out": nc.dram_tensor("out", [2, 128, 48 * FW],
                                  mybir.dt.bfloat16,
                                  kind="ExternalOutput").ap()}
    with tile.TileContext(nc) as tc:
        device_kernel(tc, outs, ins)
    nc.compile()
    return nc


def _get_program():
    if "nc" not in _CACHE:
        _CACHE["nc"] = _build_program()
    return _CACHE["nc"]


def _core_in_map(hp, core):
    import ml_dtypes
    bf16 = ml_dtypes.bfloat16
    cs = core_slices(hp, core)

    w1t = np.stack([hp["W1"][:, :128].T, hp["W1"][:, 128:].T],
                   axis=1)                            # [128, 2, 64]
    ke1t = np.zeros((Cc, 9, Cc), dtype=np.float32)
    ke2t = np.zeros((Cc, 9, CK), dtype=np.float32)
    for ky in range(3):
        for kx in range(3):
            t = ky * 3 + kx
            ke1t[:, t, :] = hp["W2"][:, :, ky, kx].T
            ke2t[:, t, :] = hp["W3"][:, :, ky, kx].T
    cm2 = hp["cm2"].transpose(2, 0, 1, 3).reshape(CK, 10, 72)
    # contract-128 proj weights: row up*64+m <-> cin = (2g+up)*64 + m
    projt = np.zeros((128, 2, 2, 128), dtype=np.float32)
    for g in range(2):
        for up in range(2):
            u = 2 * g + up
            for mh in range(2):
                projt[up * 64:(up + 1) * 64, g, mh, :] = \
                    hp["P"][mh * 128:(mh + 1) * 128,
                            u * 64:(u + 1) * 64].T
    gden = np.zeros((CK, 4), dtype=np.float32)
    for u in range(4):
        gden[u * 25:(u + 1) * 25, u] = 1.0
    g2 = np.ascontiguousarray(gden.T)
    bias2 = np.zeros((128, 2, 1), dtype=np.float32)
    bias2[:, 0, 0] = hp["bias2"][:128]
    bias2[:, 1, 0] = hp["bias2"][128:]
    qa = np.zeros((CK, 2), dtype=np.float32)
    qa[:, 0] = 0.25 * (1.0 - cs["e_top"])
    qa[:, 1] = 0.75 + 0.25 * cs["e_top"]
    ebot = np.zeros((CK, 2), dtype=np.float32)
    ebot[:, 0] = cs["e_bot"]
    ebot[:, 1] = 1.0 - cs["e_bot"]
    mc = np.concatenate([cs["mask_comp"][0:4], cs["mask_comp"][16:20]])
    mk = np.concatenate([cs["mask_k1"][0:2], cs["mask_k1"][14:16]])
    mask1 = np.broadcast_to(mc[None, :, None], (Cc, 8, 52)).copy()
    mask2 = np.broadcast_to(mk[None, :, None], (Cc, 4, 50)).copy()

    return {
        "xin": np.ascontiguousarray(cs["xin"].transpose(1, 0, 2, 3)).astype(bf16),
        "w1t": w1t.astype(bf16),
        "ke1t": ke1t.astype(bf16),
        "ke2t": ke2t.astype(bf16),
        "cm2": np.ascontiguousarray(cm2).astype(bf16),
        "projt": projt.astype(bf16),
        "gden": gden.astype(bf16),
        "g2": g2.astype(bf16),
        "srep": hp["srep"].astype(bf16),
        "sidx": hp["sidx"],
        "xt3": cs["xt3"].astype(bf16),
        "mask1": mask1.astype(bf16),
        "mask2": mask2.astype(bf16),
        "bias1": np.ascontiguousarray(hp["bias1"][:, None]),
        "biask": np.ascontiguousarray(hp["biask"][:, None]),
        "bias2": bias2,
        "qa": qa,
        "ebot": ebot,
    }


def _gather(results):
    out = np.zeros((N, C, FH, FW), dtype=np.float32)
    for core in range(NCORES):
        n, blk = core // 4, core % 4
        a0 = blk * BLK
        # device layout [m][c][pair][u1][a][p][side][q][bl][u2]:
        # row = (pair, a, p, u1), col = 96*side + 4*bl + 2*q + u2
        o = results[core]["out"].astype(np.float32).reshape(
            2, 128, 6, 2, 2, 2, 2, 2, 24, 2)
        o = o.transpose(0, 1, 2, 4, 5, 3, 6, 8, 7, 9).reshape(
            2, 128, 48, FW)
        out[n, :128, 4 * a0: 4 * a0 + 48, :] = o[0]
        out[n, 128:, 4 * a0: 4 * a0 + 48, :] = o[1]
    return out


def kernel(**inputs) -> np.ndarray:
    if os.environ.get("CARAFE_EMULATE"):
        return emulate(inputs)
    from concourse.bass_utils import run_bass_kernel_spmd
    hp = host_prep(inputs)
    nc = _get_program()
    in_maps = [_core_in_map(hp, core) for core in range(NCORES)]
    res = run_bass_kernel_spmd(nc, in_maps, list(range(NCORES)),
                               trace=bool(os.environ.get("CARAFE_TRACE")))
    _CACHE["last_results"] = res
    return _gather(res.results)



# revision 82
# speedup vs baseline: 1.0157x; 1.0035x over previous
"""
ContentAwareUpsampling (CARAFE-style) Trainium2 Bass kernel.

x[2,256,48,48] -> out[2,256,192,192]; 8 cores = 2 batches x 4 blocks of 12 rows.

Decomposition (validated bit-close by emulate() against the jax reference):
  branch convs (bf16 matmuls, BN+relu fused into ACT drains)
  softmax via exp (ACT) + group-sum/broadcast matmuls + reciprocal
  T = collapse(bilinear(softmax)) as 9 shifted matmuls vs constant [100,144]
  band matrices B built by GPSIMD local_scatter with constant indices
  einsum  E[c,(u,q,b)] = sum_dy xT[row a+dy].T @ B[dy]     (per row a, parity p)
  proj    F[r] = sum_u P_sub[u].T @ E_sigma[r*64:+64]      (sigma baked in xt)
  pixel-shuffle + BN + relu fused into the ACT drains; DMA out.
"""

import os
import numpy as np

# ---------------------------------------------------------------- constants
N, C, H, W = 2, 256, 48, 48
Cc, CK, UP, K5 = 64, 100, 2, 5
EPS = 1e-5
NCORES = 8
BLK = 12            # lo-res rows per core block
R_XIN = 20          # x rows for branch (a0-4 .. a0+15)
R_XT = 14           # x rows for einsum (a0-1 .. a0+12)
R_KN = 14           # kernel rows (a0-1 .. a0+12)
OH, OW = H * UP, W * UP          # 96
FH, FW = OH * UP, OW * UP        # 192

DYMAP = {0: [-1, -1, 0, 0, 1], 1: [-1, 0, 0, 1, 1]}
BLEND = {0: {-1: 0.25, 0: 0.75}, 1: {0: 0.75, 1: 0.25}}


def _f32(a):
    return np.ascontiguousarray(a, dtype=np.float32)


# ---------------------------------------------------------------- host prep
def _fold_bn(w, g, b, m, v):
    inv = g / np.sqrt(v + EPS)
    return w * inv[:, None], b - m * inv


def build_c2mats():
    """C2[(rho+1)*5 + (s+2)] [100, 144]: D2[b', (p,dy,q,u,dx)] =
    sum_{rho,s} Kn[:, j+rho+1, b'+s+2].T @ C2, with tau = s + dx."""
    cm = np.zeros((15, CK, 144), dtype=np.float32)
    for p in range(2):
        for q in range(2):
            for u in range(4):
                for ky in range(K5):
                    dy = DYMAP[p][ky]
                    for kx in range(K5):
                        dx = DYMAP[q][kx]
                        col = p * 72 + (dy + 1) * 24 + q * 12 + u * 3 + (dx + 1)
                        row = u * 25 + ky * 5 + kx
                        for rho, br in BLEND[p].items():
                            for tau, bc in BLEND[q].items():
                                s = tau - dx
                                w = (rho + 1) * 5 + (s + 2)
                                cm[w, row, col] += br * bc
    return cm


def build_cm2():
    """cm2[p, s'] [100, 72]: D2_p[b', (dy,q,u,dx)] =
    sum_{s'} Kb_p[:, j, b'+s'].T @ cm2[p, s'], where Kb_p holds the
    rho-blended kernel rows (blend folded on DVE) and tau = s' - 2 + dx."""
    cm = np.zeros((2, 5, CK, 72), dtype=np.float32)
    for p in range(2):
        for q in range(2):
            for u in range(4):
                for ky in range(K5):
                    dy = DYMAP[p][ky]
                    for kx in range(K5):
                        dx = DYMAP[q][kx]
                        col = (dy + 1) * 24 + q * 12 + u * 3 + (dx + 1)
                        row = u * 25 + ky * 5 + kx
                        for tau, bc in BLEND[q].items():
                            s = tau - dx
                            cm[p, s + 2, row, col] += bc
    return cm


# einsum-u -> B column block. Block k = up*2 + g where u = 2g + up, so the
# PSUM einsum output comes out ordered (up, g, q, b) and the proj repack can
# slice by u-parity with plain partition-offset copies.
U2BLK = [0, 2, 1, 3]


def build_scat_idx():
    """Scatter index set for one [80, 192] B half-tile per (j, p, side).
    Partition rho = dy*26 + w holds x column (cbase + w - 1); its source
    data is the dy-replicated D2 p-block [80, 72] with cols (dy', q, u,
    dx'); entries with dy' != dy (or out-of-window b) are -1 (ignored).
    Output cols are (ublk4, q2, bl24) with bl = w - 1 - dx."""
    sidx = np.full((80, 72), -1, dtype=np.int16)
    for dy in range(3):
        for w in range(26):
            rho = dy * 26 + w
            for q in range(2):
                for u in range(4):
                    for dx_i in range(3):
                        t = dy * 24 + q * 12 + u * 3 + dx_i
                        bl = w - dx_i
                        if 0 <= bl < 24:
                            sidx[rho, t] = U2BLK[u] * 48 + q * 24 + bl
    return sidx


def host_prep(inputs):
    x = _f32(inputs["x"])
    W1, bias1 = _fold_bn(_f32(inputs["compress_w"])[:, :, 0, 0],
                         _f32(inputs["g1"]), _f32(inputs["b1"]),
                         _f32(inputs["m1"]), _f32(inputs["v1"]))
    invk = _f32(inputs["gk"]) / np.sqrt(_f32(inputs["vk"]) + EPS)
    W2 = _f32(inputs["ke1_w"]) * invk[:, None, None, None]
    biask = _f32(inputs["bk"]) - _f32(inputs["mk"]) * invk
    W3 = _f32(inputs["ke2_w"])
    P, bias2 = _fold_bn(_f32(inputs["proj_w"])[:, :, 0, 0],
                        _f32(inputs["g2"]), _f32(inputs["b2"]),
                        _f32(inputs["m2"]), _f32(inputs["v2"]))
    cmats = build_c2mats()
    cm2 = build_cm2()
    sidx = build_scat_idx()
    # dy-replication selector: D2rep[dy*26+w', :] = D2[w' + 24*side, :]
    srep = np.zeros((50, 2, 78), dtype=np.float32)
    for side in range(2):
        for dy in range(3):
            for w in range(26):
                srep[w + 24 * side, side, dy * 26 + w] = 1.0
    # sigma position p = r*64 + m  ->  channel c = 4m + r
    sigma = np.array([4 * (p % 64) + p // 64 for p in range(256)])
    return dict(x=x, W1=W1, bias1=bias1, W2=W2, biask=biask, W3=W3,
                P=P, bias2=bias2, cmats=cmats, cm2=cm2, sidx=sidx,
                srep=srep, sigma=sigma)


def core_slices(hp, core):
    n, blk = core // 4, core % 4
    a0 = blk * BLK
    x = hp["x"][n]

    xin = np.zeros((2, 128, R_XIN, 52), dtype=np.float32)
    for s in range(R_XIN):
        row = a0 - 4 + s
        if 0 <= row < H:
            xin[0, :, s, 2:50] = x[:128, row, :]
            xin[1, :, s, 2:50] = x[128:, row, :]

    # xt kept for the numpy emulation path only
    xs = x[hp["sigma"]]
    xt = np.zeros((50, R_XT, 2, 128), dtype=np.float32)
    for s in range(R_XT):
        row = a0 - 1 + s
        if 0 <= row < H:
            xt[1:49, s, 0, :] = xs[:128, row, :].T
            xt[1:49, s, 1, :] = xs[128:, row, :].T

    # xt3: dy-replicated band windows. Partition dy*26 + w holds x column
    # (cbase(side) + w - 1) of lo-res row a0 + j - 1 + dy (0 if OOB).
    xt3 = np.zeros((80, BLK, 2, 2, 128), dtype=np.float32)
    for dy in range(3):
        for w in range(26):
            rho = dy * 26 + w
            for j in range(BLK):
                row = a0 + j - 1 + dy
                if not (0 <= row < H):
                    continue
                for side in range(2):
                    xcol = (side * 24) + w - 1
                    if 0 <= xcol < W:
                        xt3[rho, j, side, 0, :] = xs[:128, row, xcol]
                        xt3[rho, j, side, 1, :] = xs[128:, row, xcol]

    e_top = 1.0 if a0 == 0 else 0.0
    e_bot = 1.0 if a0 + BLK == H else 0.0
    mask_comp = np.array([1.0 if 0 <= a0 - 4 + s < H else 0.0
                          for s in range(R_XIN)], dtype=np.float32)
    mask_k1 = np.array([1.0 if 0 <= a0 - 2 + s < H else 0.0
                        for s in range(16)], dtype=np.float32)
    return dict(a0=a0, n=n, xin=xin, xt=xt, xt3=xt3, e_top=e_top,
                e_bot=e_bot, mask_comp=mask_comp, mask_k1=mask_k1)


# ---------------------------------------------------------------- emulation
def emulate_core(hp, cs):
    xin, xt = cs["xin"], cs["xt"]
    W1, W2, W3 = hp["W1"], hp["W2"], hp["W3"]

    xin_flat = np.concatenate([xin[0], xin[1]], axis=0)
    comp = np.zeros((Cc, R_XIN, 52), dtype=np.float32)
    comp[:, :, 2:50] = np.maximum(
        np.einsum("oc,csw->osw", W1, xin_flat[:, :, 2:50])
        + hp["bias1"][:, None, None], 0.0)
    comp *= cs["mask_comp"][None, :, None]

    k1 = np.zeros((Cc, 16, 50), dtype=np.float32)
    acc = np.zeros((Cc, 16, 48), dtype=np.float32)
    for ky in range(3):
        for kx in range(3):
            sh = comp[:, ky * 2: ky * 2 + 16, kx * 2: kx * 2 + 48]
            acc += np.einsum("oi,isw->osw", W2[:, :, ky, kx], sh)
    k1[:, :, 1:49] = np.maximum(acc + hp["biask"][:, None, None], 0.0)
    k1 *= cs["mask_k1"][None, :, None]

    kc = np.zeros((CK, R_KN, 48), dtype=np.float32)
    for ky in range(3):
        for kx in range(3):
            kc += np.einsum("oi,isw->osw", W3[:, :, ky, kx],
                            k1[:, ky: ky + 14, kx: kx + 48])

    e = np.exp(kc.reshape(4, 25, R_KN, 48))
    kn_int = (e / e.sum(axis=1, keepdims=True)).reshape(CK, R_KN, 48)

    # Kn [100, 14, 54]: col i <-> kernel pixel b = i - 3; interior i 3..50
    Kn = np.zeros((CK, R_KN, 54), dtype=np.float32)
    Kn[:, :, 3:51] = kn_int
    Kn[:, :, 2] = Kn[:, :, 3]
    Kn[:, :, 51] = Kn[:, :, 50]
    if cs["e_top"] == 1.0:
        Kn[:, 0, :] = Kn[:, 1, :]
    if cs["e_bot"] == 1.0:
        Kn[:, 13, :] = Kn[:, 12, :]

    cmats, P = hp["cmats"], hp["P"]
    scat_idx = np.full((64, 72), -1, dtype=np.int16)
    for bp in range(50):
        for dy_i in range(3):
            for q in range(2):
                for u in range(4):
                    for dx_i in range(3):
                        t = dy_i * 24 + q * 12 + u * 3 + dx_i
                        b = bp - 1 - (dx_i - 1)
                        if 0 <= b < 48:
                            scat_idx[bp, t] = dy_i * 384 + u * 96 + q * 48 + b
    out = np.zeros((C, 4 * BLK, FW), dtype=np.float32)
    for j in range(BLK):
        # D2 [50, (p, dy, q, u, dx)]
        D2 = np.zeros((50, 144), dtype=np.float32)
        for rho in (-1, 0, 1):
            for s in (-2, -1, 0, 1, 2):
                w = (rho + 1) * 5 + (s + 2)
                D2 += Kn[:, j + rho + 1, s + 2: s + 52].T @ cmats[w]
        for p in range(2):
            D = np.zeros((64, 72), dtype=np.float32)
            D[0:50] = D2[:, p * 72:(p + 1) * 72]
            B = np.zeros((64, 1152), dtype=np.float32)
            for bp in range(64):
                for t in range(72):
                    ix = scat_idx[bp, t]
                    if ix >= 0:
                        B[bp, ix] = D[bp, t]
            E = np.zeros((256, 384), dtype=np.float32)
            for ch in range(2):
                acc = np.zeros((128, 384), dtype=np.float32)
                for dy_i in range(3):
                    xsl = xt[0:50, j + dy_i, ch, :]
                    acc += xsl.T @ B[0:50, dy_i * 384:(dy_i + 1) * 384]
                E[ch * 128:(ch + 1) * 128] = acc
            for r in range(4):
                u1, u2 = r // 2, r % 2
                F = np.zeros((C, 96), dtype=np.float32)
                for u in range(4):
                    F += P[:, u * 64:(u + 1) * 64] @ \
                        E[r * 64:(r + 1) * 64, u * 96:(u + 1) * 96]
                F = np.maximum(F + hp["bias2"][:, None], 0.0)
                Y = 4 * j + 2 * p + u1
                Fq = F.reshape(C, 2, 48)
                for q in range(2):
                    out[:, Y, u2 + 2 * q::4] = Fq[:, q, :]
    return out


def emulate(inputs):
    hp = host_prep(inputs)
    out = np.zeros((N, C, FH, FW), dtype=np.float32)
    for core in range(NCORES):
        cs = core_slices(hp, core)
        n, a0 = cs["n"], cs["a0"]
        out[n, :, 4 * a0: 4 * a0 + 48, :] = emulate_core(hp, cs)
    return out


# ---------------------------------------------------------------- device
_CACHE = {}

INPUT_SPECS = [
    # DMA issue order == list order; w1t first so the PE's ldweights can
    # start the p-state ramp while xin is still in flight
    ("w1t", [128, 2, Cc], "bf16"),
    ("bias1", [Cc, 1], "f32"),
    ("xin", [128, 2, R_XIN, 52], "bf16"),
    ("mask1", [Cc, 8, 52], "bf16"),
    ("ke1t", [Cc, 9, Cc], "bf16"),
    ("biask", [Cc, 1], "f32"),
    ("mask2", [Cc, 4, 50], "bf16"),
    ("ke2t", [Cc, 9, CK], "bf16"),
    ("gden", [CK, 4], "bf16"),
    ("g2", [4, CK], "bf16"),
    ("qa", [CK, 2], "f32"),
    ("ebot", [CK, 2], "f32"),
    ("cm2", [CK, 10, 72], "bf16"),
    ("srep", [50, 2, 78], "bf16"),
    ("sidx", [80, 72], "i16"),
    ("xt3", [80, BLK, 2, 2, 128], "bf16"),
    ("bias2", [128, 2, 1], "f32"),
    ("projt", [128, 2, 2, 128], "bf16"),
]


def device_kernel(tc, outs, ins):
    """Emit the kernel. outs/ins: dicts name -> bass.AP (DRAM)."""
    from contextlib import ExitStack
    import concourse.bass as bass
    from concourse import mybir

    f32 = mybir.dt.float32
    bf16 = mybir.dt.bfloat16
    i16 = mybir.dt.int16
    AF = mybir.ActivationFunctionType
    ALU = mybir.AluOpType
    nc = tc.nc
    d_out = outs["out"]

    with ExitStack() as ctx:
        sing = ctx.enter_context(tc.tile_pool(name="sing", bufs=1))

        # ---- load inputs/constants (critical-path first, bulky ones later)
        sb = {}
        dts = {"bf16": bf16, "f32": f32, "i16": i16}
        for name, shape, dt in INPUT_SPECS:
            t = sing.tile(shape[:], dts[dt], tag=name)
            sb[name] = t
            if name == "xt3":
                nc.sync.dma_start(t[:, 0:2], ins[name][:, 0:2])
                nc.sync.dma_start(t[:, 2:6], ins[name][:, 2:6])
                nc.sync.dma_start(t[:, 6:12], ins[name][:, 6:12])
            elif name == "xin":
                # split by c-half: conv1's first matmul only needs c=0
                nc.sync.dma_start(t[:, 0], ins[name][:, 0])
                nc.sync.dma_start(t[:, 1], ins[name][:, 1])
            else:
                nc.sync.dma_start(t[:], ins[name])

        # persistent activations. All memsets go to the (otherwise idle at
        # start) GPSIMD engine, and only cover the pad regions the compute
        # never writes.
        dsbs = [sing.tile([80, 144], bf16, name=f"dsbr{i}") for i in range(4)]
        for t in dsbs:
            nc.gpsimd.memset(t[:], 0.0)
        dsb50s = [sing.tile([50, 144], bf16, name=f"dsb50r{i}")
                  for i in range(4)]
        comp = sing.tile([Cc, R_XIN, 52], bf16)
        k1 = sing.tile([Cc, 16, 50], bf16)
        exps = sing.tile([CK, R_KN, 48], bf16)
        recs = sing.tile([4, R_KN, 48], bf16)
        kn = sing.tile([CK, R_KN, 54], bf16)
        kb = sing.tile([CK, 2, BLK, 54], bf16)
        t75 = sing.tile([CK, BLK, 54], bf16)
        tb = sing.tile([CK, 54], bf16)
        nc.gpsimd.memset(comp[:, :, 0:2], 0.0)
        nc.gpsimd.memset(comp[:, :, 50:52], 0.0)
        nc.gpsimd.memset(k1[:, :, 0:1], 0.0)
        nc.gpsimd.memset(k1[:, :, 49:50], 0.0)
        nc.gpsimd.memset(kn[:, :, 0:3], 0.0)
        nc.gpsimd.memset(kn[:, :, 51:54], 0.0)

        def emit_kb(jr):
            # rho-blend fold: kb[p=0,j] = .25 kn[j] + .75 kn[j+1]
            #                 kb[p=1,j] = .75 kn[j+1] + .25 kn[j+2]
            lo, hi = jr.start, jr.stop
            nc.scalar.mul(t75[:, jr, :], kn[:, lo + 1:hi + 1, :], 0.75)
            nc.vector.scalar_tensor_tensor(
                kb[:, 0, jr, :], kn[:, lo:hi, :], 0.25,
                t75[:, jr, :], op0=ALU.mult, op1=ALU.add)
            nc.vector.scalar_tensor_tensor(
                kb[:, 1, jr, :], kn[:, lo + 2:hi + 2, :], 0.25,
                t75[:, jr, :], op0=ALU.mult, op1=ALU.add)

        # pools used by emit_tdb (created early; 1 PSUM bank + branch's 6)
        bpool = ctx.enter_context(tc.tile_pool(name="bb", bufs=6))
        pp_t = ctx.enter_context(tc.tile_pool(name="pp_t", bufs=1, space="PSUM"))
        bts = {}

        def emit_t(j):
            # T collapse for row j: D2 [50, 144] -> bf16 dsb50 ring
            pst = pp_t.tile([50, 144], f32, tag="t", name=f"pst{j}")
            for p in range(2):
                for s in range(5):
                    nc.tensor.matmul(
                        pst[:, p * 72:(p + 1) * 72], kb[:, p, j, s: s + 50],
                        sb["cm2"][:, p * 5 + s, :],
                        start=(s == 0), stop=(s == 4))
            nc.scalar.copy(dsb50s[j % 4][:], pst[:])

        def emit_b(j):
            # dy-replication via a constant 0/1 selection matmul (the
            # matmul weights AP must be 1-D, so a [dy3, w26] kb slice is
            # not allowed), then one band scatter per (p, side)
            psr = pp_t.tile([78, 2, 144], f32, tag="r", name=f"psr{j}")
            for side in range(2):
                nc.tensor.matmul(psr[:, side, :], sb["srep"][:, side, :],
                                 dsb50s[j % 4][:], start=True, stop=True)
                dsb = dsbs[(2 * j + side) % 4]
                if side == 0:
                    nc.scalar.copy(dsb[0:78, :], psr[:, side, :])
                else:
                    nc.vector.tensor_copy(dsb[0:78, :], psr[:, side, :])
                for p in range(2):
                    bt = bpool.tile([80, 192], bf16, tag="b",
                                    name=f"bt{j}_{p}_{side}")
                    nc.gpsimd.local_scatter(
                        bt[:], dsb[:, p * 72:(p + 1) * 72],
                        sb["sidx"][:], channels=80,
                        num_elems=192, num_idxs=72)
                    bts[(j, p, side)] = bt

        def emit_tdb(j):
            emit_t(j)
            emit_b(j)

        # ---- branch phase
        # chunked so early rows flow to the main loop ASAP:
        #   comp rows 0:7 | 7:14 | 14:20 ; k1 rows 0:10 | 10:16 ;
        #   kn rows 0:8 | 8:14
        with tc.tile_pool(name="brp", bufs=2, space="PSUM") as brp:
            for h, (c0, CL) in enumerate(((0, 7), (7, 7), (14, 6))):
                ps = brp.tile([Cc, CL * 48], f32, tag="psc")
                for c in range(2):
                    nc.tensor.matmul(
                        ps[:], sb["w1t"][:, c, :],
                        sb["xin"][:, c, c0:c0 + CL, 2:50],
                        start=(c == 0), stop=(c == 1))
                rs = slice(c0, c0 + CL)
                if h == 0:
                    nc.vector.tensor_scalar(
                        comp[:, rs, 2:50],
                        ps[:].rearrange("c (r w) -> c r w", w=48),
                        sb["bias1"][:], 0.0, op0=ALU.add, op1=ALU.max)
                else:
                    nc.scalar.activation(
                        comp[:, rs, 2:50],
                        ps[:].rearrange("c (r w) -> c r w", w=48),
                        AF.Relu, bias=sb["bias1"][:])
                # only rows 0:4 (top core) / 16:20 (bottom core) can be OOB
                if h == 0:
                    nc.vector.tensor_mul(comp[:, 0:4, :], comp[:, 0:4, :],
                                         sb["mask1"][:, 0:4, :])
                elif h == 2:
                    nc.vector.tensor_mul(comp[:, 16:20, :],
                                         comp[:, 16:20, :],
                                         sb["mask1"][:, 4:8, :])
            for r0, L in ((0, 10), (10, 6)):
                ps = brp.tile([Cc, L * 48], f32, tag="psk1")
                for ky in range(3):
                    for kx in range(3):
                        t = ky * 3 + kx
                        nc.tensor.matmul(
                            ps[:], sb["ke1t"][:, t, :],
                            comp[:, r0 + ky * 2: r0 + ky * 2 + L,
                                 kx * 2: kx * 2 + 48],
                            start=(t == 0), stop=(t == 8))
                rs = slice(r0, r0 + L)
                nc.vector.tensor_scalar(
                    k1[:, rs, 1:49],
                    ps[:].rearrange("c (r w) -> c r w", w=48),
                    sb["biask"][:], 0.0, op0=ALU.add, op1=ALU.max)
                # only rows 0:2 (top core) / 14:16 (bottom core) can be OOB
                mr = slice(0, 2) if r0 == 0 else slice(14, 16)
                ms = slice(0, 2) if r0 == 0 else slice(2, 4)
                nc.vector.tensor_mul(k1[:, mr, :], k1[:, mr, :],
                                     sb["mask2"][:, ms, :])
            for ci, (r0, L) in enumerate(((0, 8), (8, 6))):
                rs = slice(r0, r0 + L)
                ps = brp.tile([CK, L * 48], f32, tag="psk2", bufs=2)
                for ky in range(3):
                    for kx in range(3):
                        t = ky * 3 + kx
                        nc.tensor.matmul(
                            ps[:], sb["ke2t"][:, t, :],
                            k1[:, r0 + ky: r0 + ky + L, kx: kx + 48],
                            start=(t == 0), stop=(t == 8))
                nc.scalar.activation(
                    exps[:, rs, :],
                    ps[:].rearrange("c (r w) -> c r w", w=48), AF.Exp)
                psd = brp.tile([4, L * 48], f32, tag="psc")
                nc.tensor.matmul(psd[:], sb["gden"][:], exps[:, rs, :],
                                 start=True, stop=True)
                with nc.allow_low_precision(reason="bf16 softmax pipeline"):
                    nc.vector.reciprocal(
                        recs[:, rs, :],
                        psd[:].rearrange("c (r w) -> c r w", w=48))
                psb = brp.tile([CK, L * 48], f32, tag="psc")
                nc.tensor.matmul(psb[:], sb["g2"][:], recs[:, rs, :],
                                 start=True, stop=True)
                nc.vector.tensor_mul(
                    kn[:, rs, 3:51], exps[:, rs, :],
                    psb[:].rearrange("c (r w) -> c r w", w=48))
                nc.vector.tensor_copy(kn[:, rs, 2:3], kn[:, rs, 3:4])
                nc.vector.tensor_copy(kn[:, rs, 51:52], kn[:, rs, 50:51])
                if ci == 0:
                    # kb rows 0/1 with the top-edge row blend folded into
                    # row 0's coefficients: kb[0,0] = a*kn0 + b*kn1 where
                    # a = .25*(1-e_top), b = .75 + .25*e_top
                    nc.scalar.mul(t75[:, 0:2, :], kn[:, 1:3, :], 0.75)
                    nc.scalar.mul(tb[:], kn[:, 1, :], sb["qa"][:, 1:2])
                    nc.vector.scalar_tensor_tensor(
                        kb[:, 0, 0, :], kn[:, 0, :], sb["qa"][:, 0:1],
                        tb[:], op0=ALU.mult, op1=ALU.add)
                    nc.vector.scalar_tensor_tensor(
                        kb[:, 0, 1, :], kn[:, 1, :], 0.25,
                        t75[:, 1, :], op0=ALU.mult, op1=ALU.add)
                    nc.vector.scalar_tensor_tensor(
                        kb[:, 1, 0:2, :], kn[:, 2:4, :], 0.25,
                        t75[:, 0:2, :], op0=ALU.mult, op1=ALU.add)
                    emit_kb(slice(1, 6))
                    # start the T chains for the first rows while the
                    # rest of the branch still runs
                    emit_t(0)
                    emit_t(1)
                    emit_t(2)
                    emit_t(3)
                    emit_b(0)
                    emit_b(1)
                else:
                    nc.vector.tensor_scalar_mul(kn[:, 13, :], kn[:, 13, :],
                                                sb["ebot"][:, 1:2])
                    nc.vector.scalar_tensor_tensor(
                        kn[:, 13, :], kn[:, 12, :], sb["ebot"][:, 0:1],
                        kn[:, 13, :], op0=ALU.mult, op1=ALU.add)

        # ---- main loop
        epool = ctx.enter_context(tc.tile_pool(name="esb", bufs=3))
        eapool = ctx.enter_context(tc.tile_pool(name="ea", bufs=2))
        spool = ctx.enter_context(tc.tile_pool(name="stage", bufs=3))
        pp_e = ctx.enter_context(tc.tile_pool(name="pp_e", bufs=2, space="PSUM"))
        pp_f = ctx.enter_context(tc.tile_pool(name="pp_f", bufs=2, space="PSUM"))

        esb = {}
        stage = {}

        emit_kb(slice(6, 12))
        for apair in range(BLK // 2):
            j0 = 2 * apair
            # esb: [c-half, a2, up2, g2, p2, q2, b48]
            # esb: [c-half, a2, up2, g2, p2, side2, q2, bl24]
            esb = {c: epool.tile([128, 2, 2, 2, 2, 2, 2, 24], bf16,
                                 tag=f"e{c}", name=f"esb{c}")
                   for c in range(2)}
            # [c, u1, a2, p, b, q, u2]: u1 outermost so each DMA
            # slice is one contiguous 768-element run per partition
            # stage: [c, u1, a2, p2, side2, q2, bl24, u2]
            stage = {m: spool.tile([128, 2, 2, 2, 2, 2, 24, 2], bf16,
                                   tag=f"s{m}", name=f"stage{m}")
                     for m in range(2)}
            for j in (j0, j0 + 1):
                btp = {(p, side): bts.pop((j, p, side))
                       for p in range(2) for side in range(2)}
                for c in range(2):
                    # one PSUM tile covers both parities -> one wide drain.
                    # Each (p, side) matmul writes a contiguous 192-col
                    # region (matmul PSUM outputs must not be strided);
                    # the drain's AP permutation restores esb's layout.
                    pse = pp_e.tile([128, 2, 512], f32, tag="e")
                    for p in range(2):
                        for side in range(2):
                            nc.tensor.matmul(
                                pse[:, p, side * 192:side * 192 + 192],
                                sb["xt3"][0:78, j, side, c, :],
                                btp[(p, side)][0:78, :],
                                start=True, stop=True)
                    if c == 0:
                        # DVE's table-driven lowering accepts the 4-D
                        # pattern; one wide drain
                        nc.vector.tensor_copy(
                            esb[c][:, j % 2],
                            pse[:, :, 0:384].rearrange(
                                "c p (side up g q bl) -> c up g p side q bl",
                                side=2, up=2, g=2, q=2))
                    else:
                        # ACT codegen is limited to 3-D free patterns;
                        # split by parity
                        for p in range(2):
                            nc.scalar.copy(
                                esb[c][:, j % 2, :, :, p],
                                pse[:, p, 0:384].rearrange(
                                    "c (side up g q bl) -> c up g side q bl",
                                    side=2, up=2, g=2, q=2))
                # staggered T/B prefetch: B for j+2 (whose T ran last
                # apair, so its drain has landed), T for j+4. Keeps
                # independent PE runway between einsum and proj.
                if j + 2 < BLK:
                    emit_b(j + 2)
                if j + 4 < BLK:
                    emit_t(j + 4)

            # repack + proj, r-interleaved: proj(r) starts as soon as its
            # own two repack copies land. eA[r] rows up*64+m hold channel
            # (4m+r) at einsum-u = 2g+up (g in the free dim); the copies
            # are plain partition-offset moves that DVE runs in 4x mode.
            for r in range(4):
                ea = eapool.tile([128, 2, 2, 2, 2, 2, 24], bf16,
                                 tag=f"ea{r}", name=f"ea{r}_{apair}")
                c, h = r // 2, r % 2
                for up in range(2):
                    nc.vector.tensor_copy(
                        ea[up * 64:(up + 1) * 64],
                        esb[c][h * 64:(h + 1) * 64, :, up])
                u1, u2 = r // 2, r % 2
                for m in range(2):
                    psf = pp_f.tile([128, 2 * 192], f32, tag="f")
                    for g in range(2):
                        nc.tensor.matmul(
                            psf[:], sb["projt"][:, g, m, :],
                            ea[:, :, g, :, :, :],
                            start=(g == 0), stop=(g == 1))
                    src = psf[:].rearrange(
                        "c (a p side q bl) -> c a p side q bl",
                        a=2, p=2, side=2, q=2)
                    dst = stage[m][:, u1, :, :, :, :, :, u2]
                    if not (m == 1 and r % 2 == 1):
                        nc.scalar.activation(dst, src, AF.Relu,
                                             bias=sb["bias2"][:, m, :])
                    else:
                        nc.vector.tensor_scalar(
                            dst, src, sb["bias2"][:, m, :], 0.0,
                            op0=ALU.add, op1=ALU.max)
                    if r % 2 == 1:
                        # u1 slice complete -> DMA it out. u1 is the
                        # outermost stage dim, so later r-writes land
                        # in a disjoint byte range (no false WAR).
                        # DRAM layout [m][c][pair][u1][a2][p][col]
                        # (bf16, unshuffled on host) keeps descriptors
                        # contiguous.
                        u1x = r // 2
                        base2 = apair * 1536 + u1x * 768
                        dsta = bass.AP(d_out[m].tensor,
                                       d_out[m].offset + base2,
                                       [d_out[m].ap[0], [1, 768]])
                        nc.sync.dma_start(
                            dsta,
                            stage[m][:, u1x].rearrange(
                                "c a p side q bl v -> c (a p side q bl v)"))


def _build_program():
    import concourse.bacc as bacc
    import concourse.tile as tile
    from concourse import mybir

    dts = {"bf16": mybir.dt.bfloat16, "f32": mybir.dt.float32,
           "i16": mybir.dt.int16}
    nc = bacc.Bacc("TRN2", target_bir_lowering=False, debug=False,
                   num_devices=NCORES)
    ins = {}
    for name, shape, dt in INPUT_SPECS:
        ins[name] = nc.dram_tensor(name, shape, dts[dt],
                                   kind="ExternalInput").ap()
    outs = {"out": nc.dram_tensor("out", [2, 128, 48 * FW],
                                  mybir.dt.bfloat16,
                                  kind="ExternalOutput").ap()}
    with tile.TileContext(nc) as tc:
        device_kernel(tc, outs, ins)
    nc.compile()
    return nc


def _get_program():
    if "nc" not in _CACHE:
        _CACHE["nc"] = _build_program()
    return _CACHE["nc"]


def _core_in_map(hp, core):
    import ml_dtypes
    bf16 = ml_dtypes.bfloat16
    cs = core_slices(hp, core)

    w1t = np.stack([hp["W1"][:, :128].T, hp["W1"][:, 128:].T],
                   axis=1)                            # [128, 2, 64]
    ke1t = np.zeros((Cc, 9, Cc), dtype=np.float32)
    ke2t = np.zeros((Cc, 9, CK), dtype=np.float32)
    for ky in range(3):
        for kx in range(3):
            t = ky * 3 + kx
            ke1t[:, t, :] = hp["W2"][:, :, ky, kx].T
            ke2t[:, t, :] = hp["W3"][:, :, ky, kx].T
    cm2 = hp["cm2"].transpose(2, 0, 1, 3).reshape(CK, 10, 72)
    # contract-128 proj weights: row up*64+m <-> cin = (2g+up)*64 + m
    projt = np.zeros((128, 2, 2, 128), dtype=np.float32)
    for g in range(2):
        for up in range(2):
            u = 2 * g + up
            for mh in range(2):
                projt[up * 64:(up + 1) * 64, g, mh, :] = \
                    hp["P"][mh * 128:(mh + 1) * 128,
                            u * 64:(u + 1) * 64].T
    gden = np.zeros((CK, 4), dtype=np.float32)
    for u in range(4):
        gden[u * 25:(u + 1) * 25, u] = 1.0
    g2 = np.ascontiguousarray(gden.T)
    bias2 = np.zeros((128, 2, 1), dtype=np.float32)
    bias2[:, 0, 0] = hp["bias2"][:128]
    bias2[:, 1, 0] = hp["bias2"][128:]
    qa = np.zeros((CK, 2), dtype=np.float32)
    qa[:, 0] = 0.25 * (1.0 - cs["e_top"])
    qa[:, 1] = 0.75 + 0.25 * cs["e_top"]
    ebot = np.zeros((CK, 2), dtype=np.float32)
    ebot[:, 0] = cs["e_bot"]
    ebot[:, 1] = 1.0 - cs["e_bot"]
    mc = np.concatenate([cs["mask_comp"][0:4], cs["mask_comp"][16:20]])
    mk = np.concatenate([cs["mask_k1"][0:2], cs["mask_k1"][14:16]])
    mask1 = np.broadcast_to(mc[None, :, None], (Cc, 8, 52)).copy()
    mask2 = np.broadcast_to(mk[None, :, None], (Cc, 4, 50)).copy()

    return {
        "xin": np.ascontiguousarray(cs["xin"].transpose(1, 0, 2, 3)).astype(bf16),
        "w1t": w1t.astype(bf16),
        "ke1t": ke1t.astype(bf16),
        "ke2t": ke2t.astype(bf16),
        "cm2": np.ascontiguousarray(cm2).astype(bf16),
        "projt": projt.astype(bf16),
        "gden": gden.astype(bf16),
        "g2": g2.astype(bf16),
        "srep": hp["srep"].astype(bf16),
        "sidx": hp["sidx"],
        "xt3": cs["xt3"].astype(bf16),
        "mask1": mask1.astype(bf16),
        "mask2": mask2.astype(bf16),
        "bias1": np.ascontiguousarray(hp["bias1"][:, None]),
        "biask": np.ascontiguousarray(hp["biask"][:, None]),
        "bias2": bias2,
        "qa": qa,
        "ebot": ebot,
    }


def _gather(results):
    out = np.zeros((N, C, FH, FW), dtype=np.float32)
    for core in range(NCORES):
        n, blk = core // 4, core % 4
        a0 = blk * BLK
        # device layout [m][c][pair][u1][a][p][side][q][bl][u2]:
        # row = (pair, a, p, u1), col = 96*side + 4*bl + 2*q + u2
        o = results[core]["out"].astype(np.float32).reshape(
            2, 128, 6, 2, 2, 2, 2, 2, 24, 2)
        o = o.transpose(0, 1, 2, 4, 5, 3, 6, 8, 7, 9).reshape(
            2, 128, 48, FW)
        out[n, :128, 4 * a0: 4 * a0 + 48, :] = o[0]
        out[n, 128:, 4 * a0: 4 * a0 + 48, :] = o[1]
    return out


def kernel(**inputs) -> np.ndarray:
    if os.environ.get("CARAFE_EMULATE"):
        return emulate(inputs)
    from concourse.bass_utils import run_bass_kernel_spmd
    hp = host_prep(inputs)
    nc = _get_program()
    in_maps = [_core_in_map(hp, core) for core in range(NCORES)]
    res = run_bass_kernel_spmd(nc, in_maps, list(range(NCORES)),
                               trace=bool(os.environ.get("CARAFE_TRACE")))
    _CACHE["last_results"] = res
    return _gather(res.results)

